# revision 12
# baseline (speedup 1.0000x reference)
"""Trainium2 Bass kernel for EquivariantUNet block (2x GNN conv + BN + attention).

Sharding: nodes are split into 8 contiguous shards of 1024 (= 2 graphs each).
Edges are bucketed by destination-node 128-block on the host; each core owns
the edges that terminate in its shard. The per-edge gather reads a replicated
node-feature table in DRAM (indirect DMA); scatter-mean is a local one-hot
matmul accumulated in PSUM. Cross-core traffic: one 1MB AllGather (conv2
node-MLP table) + two 2KB AllReduces (global batch-norm statistics).
"""

import os
import numpy as np

import concourse.bass as bass
import concourse.mybir as mybir
import concourse.tile as tile
from concourse import bacc
from concourse.masks import make_identity

F32 = mybir.dt.float32
I32 = mybir.dt.int32
AF = mybir.ActivationFunctionType
OP = mybir.AluOpType

NCORES = 8
N = 8192           # nodes
D = 256            # feature dim
NPC = N // NCORES  # nodes per core (1024)
NBLK = NPC // 128  # 128-node blocks per core (8)
H = 8              # heads
HD = D // H        # head dim (32)
GPC = 2            # graphs per core
EPS_BN = 1e-5
EPS_DIR = 1e-8
EXP_SHIFT = 3.0    # constant softmax shift (mathematically exact)

LAST_EXEC_TIME_NS = [None]


# ----------------------------------------------------------------------------
# host-side preprocessing (sharding metadata + edge features)
# ----------------------------------------------------------------------------

def _sph_harm_np(d):
    x, y, z = d[:, 0], d[:, 1], d[:, 2]
    s3, s5, s15 = 3.0 ** 0.5, 5.0 ** 0.5, 15.0 ** 0.5
    return np.stack([
        np.ones_like(x),
        s3 * x, s3 * y, s3 * z,
        s15 * x * y, s15 * y * z, (s5 / 2.0) * (3.0 * z * z - 1.0),
        s15 * x * z, (s15 / 2.0) * (x * x - y * y),
    ], axis=1).astype(np.float32)


def _host_prep(x, edge_attr, pos, edge_index):
    row = np.asarray(edge_index[0]).astype(np.int64)
    col = np.asarray(edge_index[1]).astype(np.int64)

    rel = pos[row] - pos[col]
    elen = np.sqrt((rel * rel).sum(axis=1, keepdims=True))
    dirs = rel / (elen + EPS_DIR)
    ef_all = np.concatenate([_sph_harm_np(dirs), edge_attr.astype(np.float32)],
                            axis=1)  # [E, 12]

    blk = col // 128  # global destination 128-block, 0..63
    order = np.argsort(blk, kind="stable")
    cnt_blk = np.bincount(blk, minlength=64)
    T = int(np.ceil(cnt_blk.max() / 128))  # tiles per block (uniform, SPMD)
    if (NBLK * T * 128) % 512 != 0:  # always true (1024*T % 512 == 0)
        T += 1
    Em = NBLK * T * 128

    row_s = np.zeros((NCORES, Em), np.int32)
    lcol_s = np.full((NCORES, Em), -1.0, np.float32)
    ef_s = np.zeros((NCORES, 12, Em), np.float32)
    starts = np.zeros(65, np.int64)
    starts[1:] = np.cumsum(cnt_blk)
    for g in range(64):
        c, b = g // NBLK, g % NBLK
        e_ids = order[starts[g]:starts[g + 1]]
        k = e_ids.size
        base = b * T * 128
        row_s[c, base:base + k] = row[e_ids]
        lcol_s[c, base:base + k] = (col[e_ids] - g * 128).astype(np.float32)
        ef_s[c, :, base:base + k] = ef_all[e_ids].T

    rowT = np.ascontiguousarray(
        row_s.reshape(NCORES, Em // 128, 128).transpose(0, 2, 1))
    lcolT = np.ascontiguousarray(
        lcol_s.reshape(NCORES, Em // 128, 128).transpose(0, 2, 1))

    cnt_node = np.bincount(col, minlength=N).astype(np.float32)
    inv_cnt = (1.0 / np.maximum(cnt_node, 1.0)).reshape(NCORES, NBLK, 128)
    invc = np.ascontiguousarray(inv_cnt.transpose(0, 2, 1))

    xT = np.ascontiguousarray(x.T)  # [256, 8192]
    x_ownT = np.ascontiguousarray(
        xT.reshape(D, NCORES, NPC).transpose(1, 0, 2))
    return ef_s, rowT, lcolT, invc, xT, x_ownT, T, Em


# ----------------------------------------------------------------------------
# device kernel
# ----------------------------------------------------------------------------

def _build(T, Em):
    nc = bacc.Bacc(None, target_bir_lowering=False, debug=False,
                   num_devices=NCORES)
    NT = Em // 128
    NCH = Em // 512

    d_xT = nc.dram_tensor("xT", [D, N], F32, kind="ExternalInput")
    d_xoT = nc.dram_tensor("x_ownT", [D, NPC], F32, kind="ExternalInput")
    d_efT = nc.dram_tensor("efT", [12, Em], F32, kind="ExternalInput")
    d_rowT = nc.dram_tensor("rowT", [128, NT], I32, kind="ExternalInput")
    d_lcolT = nc.dram_tensor("lcolT", [128, NT], F32, kind="ExternalInput")
    d_invc = nc.dram_tensor("invc", [128, NBLK], F32, kind="ExternalInput")

    wnames = []
    for cv in ("c1", "c2"):
        wnames += [f"{cv}_node_w1", f"{cv}_node_w2", f"{cv}_edge_w2",
                   f"{cv}_out_w1", f"{cv}_out_w2"]
    wnames += ["q_w", "k_w", "v_w", "out_w"]
    d_w = {}
    for nm in wnames:
        rows = 512 if nm.endswith("out_w1") else 256
        d_w[nm] = nc.dram_tensor(nm, [rows, D], F32, kind="ExternalInput")
    for nm in ("c1_edge_w1", "c2_edge_w1"):
        d_w[nm] = nc.dram_tensor(nm, [12, D], F32, kind="ExternalInput")

    bcol_names, brow_names = [], []
    for cv in ("c1", "c2"):
        bcol_names += [f"{cv}_node_b1", f"{cv}_edge_b1", f"{cv}_out_b1"]
        brow_names += [f"{cv}_node_b2", f"{cv}_edge_b2", f"{cv}_out_b2",
                       f"{cv}_bn_g", f"{cv}_norm_g", f"{cv}_norm_b"]
    brow_names += ["q_b", "k_b", "v_b", "out_b"]
    d_bc = {nm: nc.dram_tensor(nm, [D, 1], F32, kind="ExternalInput")
            for nm in bcol_names}
    d_br = {nm: nc.dram_tensor(nm, [1, D], F32, kind="ExternalInput")
            for nm in brow_names}

    d_out = nc.dram_tensor("out", [NPC, D], F32, kind="ExternalOutput")
    DEBUG = bool(int(os.environ.get("KERNEL_DEBUG", "0")))
    d_dbg = {}
    if DEBUG:
        d_dbg["xt1"] = nc.dram_tensor("dbg_xt1", [NPC, D], F32,
                                      kind="ExternalOutput")
        for nm in ("agg", "h", "agg2", "h2", "y"):
            d_dbg[nm] = nc.dram_tensor(f"dbg_{nm}", [128, NBLK * D], F32,
                                       kind="ExternalOutput")
        d_dbg["stg"] = nc.dram_tensor("dbg_stg", [1, 2 * D], F32,
                                      kind="ExternalOutput")
        d_dbg["ab"] = nc.dram_tensor("dbg_ab", [2, D], F32,
                                     kind="ExternalOutput")

    d_xt1 = nc.dram_tensor("xt1_table", [N, D], F32)
    d_xt2_in = nc.dram_tensor("xt2_bounce", [NPC, D], F32)
    d_xt2 = nc.dram_tensor("xt2_table", [N, D], F32, addr_space="Shared")
    d_st_in = {cv: nc.dram_tensor(f"st_in_{cv}", [1, 2 * D], F32)
               for cv in ("c1", "c2")}
    d_st_out = {cv: nc.dram_tensor(f"st_out_{cv}", [1, 2 * D], F32,
                                   addr_space="Shared")
                for cv in ("c1", "c2")}

    RG = [list(range(NCORES))]

    with tile.TileContext(nc) as tc:
        with tc.tile_pool(name="wpool", bufs=1) as wp, \
             tc.tile_pool(name="persist", bufs=1) as pp:
            # ---- persistent SBUF ------------------------------------------
            w = {}
            for nm in wnames:
                rows = 512 if nm.endswith("out_w1") else 256
                tiles = []
                for k in range(rows // 128):
                    t = wp.tile([128, D], F32, name=f"{nm}_{k}",
                                tag=f"{nm}_{k}")
                    nc.sync.dma_start(out=t[:],
                                      in_=d_w[nm][k * 128:(k + 1) * 128, :])
                    tiles.append(t)
                w[nm] = tiles
            for nm in ("c1_edge_w1", "c2_edge_w1"):
                t = wp.tile([12, D], F32, name=nm, tag=nm)
                nc.sync.dma_start(out=t[:], in_=d_w[nm][:, :])
                w[nm] = t
            bc = {}
            for nm in bcol_names:
                tiles = []
                for k in range(2):
                    t = wp.tile([128, 1], F32, name=f"{nm}_{k}",
                                tag=f"{nm}_{k}")
                    nc.sync.dma_start(out=t[:],
                                      in_=d_bc[nm][k * 128:(k + 1) * 128, :])
                    tiles.append(t)
                bc[nm] = tiles
            br = {}
            for nm in brow_names:
                t = wp.tile([1, D], F32, name=f"br_{nm}", tag=f"br_{nm}")
                nc.sync.dma_start(out=t[:], in_=d_br[nm][:, :])
                br[nm] = t

            ident = wp.tile([128, 128], F32, name="ident", tag="ident")
            make_identity(nc, ident[:])
            iota = wp.tile([128, 128], F32, name="iota", tag="iota")
            nc.gpsimd.iota(iota[:], pattern=[[1, 128]], base=0,
                           channel_multiplier=0,
                           allow_small_or_imprecise_dtypes=True)
            ones_row = wp.tile([1, 512], F32, name="ones_row", tag="ones_row")
            nc.vector.memset(ones_row[:], 1.0)
            ones_col = wp.tile([128, 1], F32, name="ones_col", tag="ones_col")
            nc.vector.memset(ones_col[:], 1.0)
            c_eps = wp.tile([1, 1], F32, name="c_eps", tag="c_eps")
            nc.vector.memset(c_eps[:], EPS_BN)
            c_nsh = wp.tile([128, 1], F32, name="c_nsh", tag="c_nsh")
            nc.vector.memset(c_nsh[:], -EXP_SHIFT)

            t_rowT = wp.tile([128, NT], I32, name="t_rowT", tag="t_rowT")
            nc.sync.dma_start(out=t_rowT[:], in_=d_rowT[:, :])
            t_lcolT = wp.tile([128, NT], F32, name="t_lcolT", tag="t_lcolT")
            nc.sync.dma_start(out=t_lcolT[:], in_=d_lcolT[:, :])
            t_invc = wp.tile([128, NBLK], F32, name="t_invc", tag="t_invc")
            nc.sync.dma_start(out=t_invc[:], in_=d_invc[:, :])

            def ptile(nm, width=NPC):
                return pp.tile([128, width], F32, name=nm, tag=nm)

            h_own = ptile("h_own", NBLK * D)      # conv1 out, node-major
            h_ownT = [ptile(f"h_ownT{i}") for i in range(2)]
            h2_own = ptile("h2_own", NBLK * D)
            h2T = [ptile(f"h2T{i}") for i in range(2)]
            agg_all = ptile("agg_all", NBLK * D)

            # ---------------------------------------------------------------
            def l2_psum(ps, htiles, w2tiles, bias_row, mslice, tag, bufs=2):
                p = ps.tile([128, D], F32, tag=tag, bufs=bufs, name=tag)
                for k in range(len(htiles)):
                    nc.tensor.matmul(p[:], htiles[k][:, mslice], w2tiles[k][:],
                                     start=(k == 0), stop=False)
                nc.tensor.matmul(p[:], ones_row[:, 0:128], bias_row[:],
                                 start=False, stop=True)
                return p

            def transpose_128(ps, dst_slice, src_slice, tag):
                p = ps.tile([128, 128], F32, tag=tag, bufs=2, name=tag)
                nc.tensor.transpose(p[:], src_slice, ident[:])
                nc.vector.tensor_copy(dst_slice, p[:])

            # ===============================================================
            # P1: conv1 node MLP over ALL nodes (replicated) -> xt1_table
            # ===============================================================
            with tc.tile_pool(name="p1ps", bufs=1, space="PSUM") as ps1, \
                 tc.tile_pool(name="p1sb", bufs=1) as sb1:
                for ch in range(N // 512):
                    rhs = []
                    for k in range(2):
                        r = sb1.tile([128, 512], F32, tag="xchunk", bufs=4,
                                     name="xchunk")
                        nc.sync.dma_start(
                            out=r[:], in_=d_xT[k * 128:(k + 1) * 128,
                                              ch * 512:(ch + 1) * 512])
                        rhs.append(r)
                    hs = []
                    for half in range(2):
                        p = ps1.tile([128, 512], F32, tag="n1p", bufs=2,
                                     name="n1p")
                        for ki in range(2):
                            nc.tensor.matmul(
                                p[:],
                                w["c1_node_w1"][ki][:, half * 128:(half + 1) * 128],
                                rhs[ki][:], start=(ki == 0), stop=(ki == 1))
                        o = sb1.tile([128, 512], F32, tag="n1s", bufs=4,
                                     name="n1s")
                        nc.scalar.activation(o[:], p[:], AF.Silu,
                                             bias=bc["c1_node_b1"][half][:, :1])
                        hs.append(o)
                    for i4 in range(4):
                        msl = slice(i4 * 128, (i4 + 1) * 128)
                        p = l2_psum(ps1, hs, w["c1_node_w2"], br["c1_node_b2"],
                                    msl, "n1o")
                        o = sb1.tile([128, D], F32, tag="n1os", bufs=4,
                                     name="n1os")
                        nc.vector.tensor_copy(o[:], p[:])
                        r0 = ch * 512 + i4 * 128
                        nc.sync.dma_start(out=d_xt1[r0:r0 + 128, :], in_=o[:])

            # ===============================================================
            # edge pipeline (both convs)
            # ===============================================================
            agg_state = {}

            def conv_edges(cv, table, dst_tile):
                with tc.tile_pool(name=f"{cv}eps", bufs=1, space="PSUM") as ps, \
                     tc.tile_pool(name=f"{cv}esb", bufs=1) as sb:
                    for ch in range(NCH):
                        ef = sb.tile([12, 512], F32, tag="ef", bufs=4,
                                     name="ef")
                        nc.sync.dma_start(
                            out=ef[:], in_=d_efT[:, ch * 512:(ch + 1) * 512])
                        hs = []
                        for half in range(2):
                            p = ps.tile([128, 512], F32, tag="ehp", bufs=2,
                                        name="ehp")
                            nc.tensor.matmul(
                                p[:],
                                w[f"{cv}_edge_w1"][:, half * 128:(half + 1) * 128],
                                ef[:], start=True, stop=True)
                            o = sb.tile([128, 512], F32, tag="ehs", bufs=4,
                                        name="ehs")
                            nc.scalar.activation(
                                o[:], p[:], AF.Silu,
                                bias=bc[f"{cv}_edge_b1"][half][:, :1])
                            hs.append(o)
                        for i4 in range(4):
                            ti = ch * 4 + i4
                            b, tin = ti // T, ti % T
                            msl = slice(i4 * 128, (i4 + 1) * 128)
                            em_p = l2_psum(ps, hs, w[f"{cv}_edge_w2"],
                                           br[f"{cv}_edge_b2"], msl, "em")
                            xtg = sb.tile([128, D], F32, tag="xtg", bufs=6,
                                          name="xtg")
                            nc.gpsimd.indirect_dma_start(
                                out=xtg[:], out_offset=None, in_=table[:, :],
                                in_offset=bass.IndirectOffsetOnAxis(
                                    ap=t_rowT[:, ti:ti + 1], axis=0))
                            msg = sb.tile([128, D], F32, tag="msg", bufs=4,
                                          name="msg")
                            nc.vector.tensor_tensor(out=msg[:], in0=xtg[:],
                                                    in1=em_p[:], op=OP.mult)
                            oh = sb.tile([128, 128], F32, tag="oh", bufs=4,
                                         name="oh")
                            nc.vector.tensor_scalar(
                                out=oh[:], in0=iota[:],
                                scalar1=t_lcolT[:, ti:ti + 1], scalar2=None,
                                op0=OP.is_equal)
                            if tin == 0:
                                agg_state["p"] = ps.tile(
                                    [128, D], F32, tag="agg", bufs=2,
                                    name="agg")
                            nc.tensor.matmul(agg_state["p"][:], oh[:], msg[:],
                                             start=(tin == 0),
                                             stop=(tin == T - 1))
                            if tin == T - 1:
                                nc.vector.tensor_scalar(
                                    out=dst_tile[:, b * D:(b + 1) * D],
                                    in0=agg_state["p"][:],
                                    scalar1=t_invc[:, b:b + 1], scalar2=None,
                                    op0=OP.mult)

            # ===============================================================
            # out-MLP + fused double-BN + SiLU (both convs)
            # ===============================================================
            def conv_out(cv, inT_tiles, y_dst, yT_dst):
                with tc.tile_pool(name=f"{cv}ops", bufs=1, space="PSUM") as ps, \
                     tc.tile_pool(name=f"{cv}osb", bufs=1) as sb:
                    y_tiles = []
                    st_p = ps.tile([1, 2 * D], F32, tag="st", bufs=1,
                                   name="st_p")
                    for ch2 in range(NPC // 512):
                        hs = []
                        for half in range(2):
                            p = ps.tile([128, 512], F32, tag="o1p", bufs=2,
                                        name="o1p")
                            for ki in range(4):
                                nc.tensor.matmul(
                                    p[:],
                                    w[f"{cv}_out_w1"][ki][:, half * 128:(half + 1) * 128],
                                    inT_tiles[ki][:, ch2 * 512:(ch2 + 1) * 512],
                                    start=(ki == 0), stop=(ki == 3))
                            o = sb.tile([128, 512], F32, tag="o1s", bufs=4,
                                        name="o1s")
                            nc.scalar.activation(
                                o[:], p[:], AF.Silu,
                                bias=bc[f"{cv}_out_b1"][half][:, :1])
                            hs.append(o)
                        for i4 in range(4):
                            bi = ch2 * 4 + i4
                            msl = slice(i4 * 128, (i4 + 1) * 128)
                            p = l2_psum(ps, hs, w[f"{cv}_out_w2"],
                                        br[f"{cv}_out_b2"], msl, "o2")
                            yy = sb.tile([128, 2 * D], F32, tag="yy",
                                         bufs=NBLK + 1, name="yy")
                            nc.vector.tensor_copy(yy[:, 0:D], p[:])
                            nc.vector.tensor_tensor(out=yy[:, D:2 * D],
                                                    in0=yy[:, 0:D],
                                                    in1=yy[:, 0:D],
                                                    op=OP.mult)
                            y_tiles.append(yy)
                            nc.tensor.matmul(st_p[:], ones_col[:], yy[:],
                                             start=(bi == 0),
                                             stop=(bi == NBLK - 1))
                    st_s = sb.tile([1, 2 * D], F32, tag="st_s", name="st_s")
                    nc.vector.tensor_copy(st_s[:], st_p[:])
                    nc.sync.dma_start(out=d_st_in[cv][:, :], in_=st_s[:])
                    nc.gpsimd.collective_compute(
                        "AllReduce", OP.add, replica_groups=RG,
                        ins=[d_st_in[cv].ap()], outs=[d_st_out[cv].ap()])
                    stg = sb.tile([1, 2 * D], F32, tag="stg", name="stg")
                    nc.sync.dma_start(out=stg[:], in_=d_st_out[cv][:, :])

                    def stile(nm):
                        return sb.tile([1, D], F32, tag=nm, name=nm)

                    mu, var, tmp = stile("mu"), stile("var"), stile("tmp")
                    nc.vector.tensor_scalar(out=mu[:], in0=stg[0:1, 0:D],
                                            scalar1=1.0 / N, scalar2=None,
                                            op0=OP.mult)
                    nc.vector.tensor_scalar(out=var[:], in0=stg[0:1, D:2 * D],
                                            scalar1=1.0 / N, scalar2=None,
                                            op0=OP.mult)
                    nc.vector.tensor_tensor(out=tmp[:], in0=mu[:], in1=mu[:],
                                            op=OP.mult)
                    nc.vector.tensor_tensor(out=var[:], in0=var[:], in1=tmp[:],
                                            op=OP.subtract)
                    sq, r1, tt = stile("sq"), stile("r1"), stile("tt")
                    nc.scalar.activation(sq[:], var[:], AF.Sqrt, bias=c_eps[:, :1])
                    nc.vector.reciprocal(r1[:], sq[:])
                    nc.vector.tensor_tensor(out=tt[:], in0=r1[:],
                                            in1=br[f"{cv}_bn_g"][:],
                                            op=OP.mult)
                    vy, sq2, r2 = stile("vy"), stile("sq2"), stile("r2")
                    nc.vector.tensor_tensor(out=vy[:], in0=tt[:], in1=tt[:],
                                            op=OP.mult)
                    nc.vector.tensor_tensor(out=vy[:], in0=vy[:], in1=var[:],
                                            op=OP.mult)
                    nc.scalar.activation(sq2[:], vy[:], AF.Sqrt, bias=c_eps[:, :1])
                    nc.vector.reciprocal(r2[:], sq2[:])
                    Ac, Bc = stile("Ac"), stile("Bc")
                    nc.vector.tensor_tensor(out=Ac[:], in0=tt[:], in1=r2[:],
                                            op=OP.mult)
                    nc.vector.tensor_tensor(out=Ac[:], in0=Ac[:],
                                            in1=br[f"{cv}_norm_g"][:],
                                            op=OP.mult)
                    nc.vector.tensor_tensor(out=Bc[:], in0=mu[:], in1=Ac[:],
                                            op=OP.mult)
                    nc.vector.tensor_tensor(out=Bc[:],
                                            in0=br[f"{cv}_norm_b"][:],
                                            in1=Bc[:], op=OP.subtract)
                    if DEBUG and cv == "c1":
                        nc.sync.dma_start(out=d_dbg["stg"][:, :], in_=stg[:])
                        nc.sync.dma_start(out=d_dbg["ab"][0:1, :], in_=Ac[:])
                        nc.sync.dma_start(out=d_dbg["ab"][1:2, :], in_=Bc[:])
                    # broadcast A/B to 128 partitions via PE
                    Ab = sb.tile([128, D], F32, tag="Ab", name="Ab")
                    Bb = sb.tile([128, D], F32, tag="Bb", name="Bb")
                    for src, dstb in ((Ac, Ab), (Bc, Bb)):
                        pbc = ps.tile([128, D], F32, tag="abp", bufs=1,
                                      name="abp")
                        nc.tensor.matmul(pbc[:], ones_row[:, 0:128], src[:],
                                         start=True, stop=True)
                        nc.vector.tensor_copy(dstb[:], pbc[:])
                    # z = silu(y*A + B)
                    if DEBUG and cv == "c1":
                        for bi in range(NBLK):
                            nc.sync.dma_start(
                                out=d_dbg["y"][:, bi * D:(bi + 1) * D],
                                in_=y_tiles[bi][:, 0:D])
                    for bi in range(NBLK):
                        y = y_tiles[bi]
                        t1 = sb.tile([128, D], F32, tag="zt", bufs=2,
                                     name="zt")
                        nc.vector.tensor_tensor(out=t1[:], in0=y[:, 0:D],
                                                in1=Ab[:], op=OP.mult)
                        nc.vector.tensor_tensor(out=t1[:], in0=t1[:],
                                                in1=Bb[:], op=OP.add)
                        nc.scalar.activation(y_dst[:, bi * D:(bi + 1) * D],
                                             t1[:], AF.Silu)
                        for half in range(2):
                            transpose_128(
                                ps, yT_dst[half][:, bi * 128:(bi + 1) * 128],
                                y_dst[:, bi * D + half * 128:
                                      bi * D + (half + 1) * 128], "ytr")

            # ---- conv1 ----------------------------------------------------
            conv_edges("c1", d_xt1, agg_all)
            if DEBUG:
                nc.sync.dma_start(out=d_dbg["xt1"][:, :], in_=d_xt1[0:NPC, :])
                dbg_agg_sb = pp.tile([128, NBLK * D], F32, name="dbg_agg_sb",
                                     tag="dbg_agg_sb")
                nc.vector.tensor_copy(dbg_agg_sb[:], agg_all[:])
                nc.sync.dma_start(out=d_dbg["agg"][:, :], in_=dbg_agg_sb[:])
            aggT = [ptile(f"aggT{i}") for i in range(2)]
            xoT = [ptile(f"xoT{i}") for i in range(2)]
            with tc.tile_pool(name="c1tps", bufs=1, space="PSUM") as pst:
                for bi in range(NBLK):
                    for half in range(2):
                        transpose_128(
                            pst, aggT[half][:, bi * 128:(bi + 1) * 128],
                            agg_all[:, bi * D + half * 128:
                                    bi * D + (half + 1) * 128], "atr")
            for half in range(2):
                nc.sync.dma_start(out=xoT[half][:],
                                  in_=d_xoT[half * 128:(half + 1) * 128, :])
            conv_out("c1", aggT + xoT, h_own, h_ownT)
            if DEBUG:
                nc.sync.dma_start(out=d_dbg["h"][:, :], in_=h_own[:])

            # ---- conv2 node MLP (own shard) + AllGather --------------------
            with tc.tile_pool(name="p4ps", bufs=1, space="PSUM") as ps4, \
                 tc.tile_pool(name="p4sb", bufs=1) as sb4:
                for ch in range(NPC // 512):
                    hs = []
                    for half in range(2):
                        p = ps4.tile([128, 512], F32, tag="n2p", bufs=2,
                                     name="n2p")
                        for ki in range(2):
                            nc.tensor.matmul(
                                p[:],
                                w["c2_node_w1"][ki][:, half * 128:(half + 1) * 128],
                                h_ownT[ki][:, ch * 512:(ch + 1) * 512],
                                start=(ki == 0), stop=(ki == 1))
                        o = sb4.tile([128, 512], F32, tag="n2s", bufs=4,
                                     name="n2s")
                        nc.scalar.activation(o[:], p[:], AF.Silu,
                                             bias=bc["c2_node_b1"][half][:, :1])
                        hs.append(o)
                    for i4 in range(4):
                        msl = slice(i4 * 128, (i4 + 1) * 128)
                        p = l2_psum(ps4, hs, w["c2_node_w2"], br["c2_node_b2"],
                                    msl, "n2o")
                        o = sb4.tile([128, D], F32, tag="n2os", bufs=4,
                                     name="n2os")
                        nc.vector.tensor_copy(o[:], p[:])
                        r0 = ch * 512 + i4 * 128
                        nc.sync.dma_start(out=d_xt2_in[r0:r0 + 128, :],
                                          in_=o[:])
                nc.gpsimd.collective_compute(
                    "AllGather", OP.bypass, replica_groups=RG,
                    ins=[d_xt2_in.ap()], outs=[d_xt2.ap()])

            # ---- conv2 ----------------------------------------------------
            conv_edges("c2", d_xt2, agg_all)
            if DEBUG:
                nc.sync.dma_start(out=d_dbg["agg2"][:, :], in_=agg_all[:])
            agg2T = [ptile(f"agg2T{i}") for i in range(2)]
            with tc.tile_pool(name="c2tps", bufs=1, space="PSUM") as pst:
                for bi in range(NBLK):
                    for half in range(2):
                        transpose_128(
                            pst, agg2T[half][:, bi * 128:(bi + 1) * 128],
                            agg_all[:, bi * D + half * 128:
                                    bi * D + (half + 1) * 128], "a2tr")
            conv_out("c2", agg2T + h_ownT, h2_own, h2T)
            if DEBUG:
                nc.sync.dma_start(out=d_dbg["h2"][:, :], in_=h2_own[:])

            # ===============================================================
            # attention (2 graphs on own shard)
            # ===============================================================
            OT = [ptile(f"OT{i}") for i in range(2)]
            with tc.tile_pool(name="apsb", bufs=1) as sbp:
              with tc.tile_pool(name="avps", bufs=1, space="PSUM") as psv:
                V = []
                for bi in range(NBLK):
                    msl = slice(bi * 128, (bi + 1) * 128)
                    p = psv.tile([128, D], F32, tag="vp", bufs=2, name="vp")
                    for ki in range(2):
                        nc.tensor.matmul(p[:], h2T[ki][:, msl],
                                         w["v_w"][ki][:],
                                         start=(ki == 0), stop=False)
                    nc.tensor.matmul(p[:], ones_row[:, 0:128], br["v_b"][:],
                                     start=False, stop=True)
                    v = sbp.tile([128, D], F32, tag=f"vsb{bi}", name=f"vsb{bi}")
                    nc.vector.tensor_copy(v[:], p[:])
                    V.append(v)

              with tc.tile_pool(name="ahps", bufs=1, space="PSUM") as psp:
                scale = 1.0 / (HD ** 0.5)
                for g in range(GPC):
                    gsl = slice(g * 512, (g + 1) * 512)
                    for h in range(H):
                        hsl = slice(h * 32, (h + 1) * 32)
                        qk = {}
                        for wn, bn in (("q_w", "q_b"), ("k_w", "k_b")):
                            p = psp.tile([32, 512], F32, tag="qkp", bufs=2,
                                         name="qkp")
                            for ki in range(2):
                                nc.tensor.matmul(p[:], w[wn][ki][:, hsl],
                                                 h2T[ki][:, gsl],
                                                 start=(ki == 0), stop=False)
                            nc.tensor.matmul(p[:], br[bn][:, hsl],
                                             ones_row[:], start=False,
                                             stop=True)
                            t = sbp.tile([32, 512], F32, tag=f"{wn}s", bufs=2,
                                         name=f"{wn}s")
                            nc.scalar.activation(t[:], p[:], AF.Copy)
                            qk[wn] = t
                        exps = []
                        for kt in range(4):
                            s_p = psp.tile([128, 512], F32, tag="sc", bufs=2,
                                           name="s_p")
                            nc.tensor.matmul(
                                s_p[:], qk["k_w"][:, kt * 128:(kt + 1) * 128],
                                qk["q_w"][:], start=True, stop=True)
                            e = sbp.tile([128, 512], F32, tag="exps", bufs=6,
                                         name="exps")
                            nc.scalar.activation(e[:], s_p[:], AF.Exp,
                                                 bias=c_nsh[:, :1],
                                                 scale=scale)
                            exps.append(e)
                        den_p = psp.tile([1, 512], F32, tag="den", bufs=1,
                                         name="den_p")
                        ut_p = psp.tile([32, 512], F32, tag="ut", bufs=2,
                                        name="ut_p")
                        for kt in range(4):
                            nc.tensor.matmul(den_p[:], ones_col[:],
                                             exps[kt][:], start=(kt == 0),
                                             stop=(kt == 3))
                            nc.tensor.matmul(
                                ut_p[:], V[g * 4 + kt][:, hsl],
                                exps[kt][:], start=(kt == 0), stop=(kt == 3))
                        rden = sbp.tile([1, 512], F32, tag="rden", bufs=2,
                                        name="rden")
                        nc.vector.reciprocal(rden[:], den_p[:])
                        rb_p = psp.tile([32, 512], F32, tag="rbp", bufs=1,
                                        name="rb_p")
                        nc.tensor.matmul(rb_p[:], ones_row[:, 0:32], rden[:],
                                         start=True, stop=True)
                        rb = sbp.tile([32, 512], F32, tag="rbs", bufs=2,
                                      name="rb")
                        nc.vector.tensor_copy(rb[:], rb_p[:])
                        ht, hr = h // 4, (h % 4) * 32
                        nc.vector.tensor_tensor(
                            out=OT[ht][hr:hr + 32, gsl],
                            in0=ut_p[:], in1=rb[:], op=OP.mult)

            with tc.tile_pool(name="aops", bufs=1, space="PSUM") as pso, \
                 tc.tile_pool(name="aosb", bufs=1) as sbo:
                for bi in range(NBLK):
                    msl = slice(bi * 128, (bi + 1) * 128)
                    p = pso.tile([128, D], F32, tag="op", bufs=2, name="op")
                    for ki in range(2):
                        nc.tensor.matmul(p[:], OT[ki][:, msl],
                                         w["out_w"][ki][:],
                                         start=(ki == 0), stop=False)
                    nc.tensor.matmul(p[:], ones_row[:, 0:128], br["out_b"][:],
                                     start=False, stop=True)
                    o = sbo.tile([128, D], F32, tag="osb", bufs=4, name="osb")
                    nc.vector.tensor_copy(o[:], p[:])
                    nc.sync.dma_start(out=d_out[bi * 128:(bi + 1) * 128, :],
                                      in_=o[:])

    nc.finalize()
    return nc


# ----------------------------------------------------------------------------
# PJRT runner (axon) with optional repeat-timing
# ----------------------------------------------------------------------------

def _run_pjrt(nc, in_maps, iters=1):
    import time as _time
    import jax
    from jax.experimental.shard_map import shard_map
    from jax.sharding import Mesh, PartitionSpec, NamedSharding
    from concourse.bass2jax import (_bass_exec_p, partition_id_tensor,
                                    install_neuronx_cc_hook)
    import concourse.mybir as mybir_

    install_neuronx_cc_hook()
    n_cores = len(in_maps)
    partition_name = (nc.partition_id_tensor.name
                      if nc.partition_id_tensor else None)
    in_names, out_names, out_avals, zero_outs = [], [], [], []
    for alloc in nc.m.functions[0].allocations:
        if not isinstance(alloc, mybir_.MemoryLocationSet):
            continue
        name = alloc.memorylocations[0].name
        if alloc.kind == "ExternalInput":
            if name != partition_name:
                in_names.append(name)
        elif alloc.kind == "ExternalOutput":
            shape = tuple(alloc.tensor_shape)
            dtype = mybir_.dt.np(alloc.dtype)
            out_names.append(name)
            out_avals.append(jax.core.ShapedArray(shape, dtype))
            zero_outs.append(np.zeros(shape, dtype))
    n_params = len(in_names)
    n_outs = len(out_avals)
    in_names_full = list(in_names) + list(out_names)
    if partition_name is not None:
        in_names_full.append(partition_name)
    donate = tuple(range(n_params, n_params + n_outs))

    def _body(*args):
        operands = list(args)
        if partition_name is not None:
            operands.append(partition_id_tensor())
        outs = _bass_exec_p.bind(
            *operands,
            out_avals=tuple(out_avals),
            in_names=tuple(in_names_full),
            out_names=tuple(out_names),
            lowering_input_output_aliases=(),
            sim_require_finite=True,
            sim_require_nnan=True,
            nc=nc,
        )
        return tuple(outs)

    devices = jax.devices()[:n_cores]
    mesh = Mesh(np.asarray(devices), ("core",))
    in_specs = (PartitionSpec("core"),) * (n_params + n_outs)
    out_specs = (PartitionSpec("core"),) * n_outs
    sharded = jax.jit(
        shard_map(_body, mesh=mesh, in_specs=in_specs, out_specs=out_specs,
                  check_rep=False),
        donate_argnums=donate, keep_unused=True)
    sh = NamedSharding(mesh, PartitionSpec("core"))
    concat_in = [
        jax.device_put(
            np.concatenate([np.asarray(in_maps[c][nm])
                            for c in range(n_cores)], axis=0), sh)
        for nm in in_names]

    def once():
        zs = [jax.device_put(
            np.zeros((n_cores * z.shape[0], *z.shape[1:]), z.dtype), sh)
            for z in zero_outs]
        return sharded(*concat_in, *zs)

    out_arrs = once()
    jax.block_until_ready(out_arrs)
    exec_ns = None
    if iters > 1:
        t0 = _time.perf_counter()
        o1 = once()
        jax.block_until_ready(o1)
        t1 = _time.perf_counter() - t0
        t0 = _time.perf_counter()
        oK = [once() for _ in range(iters)]
        jax.block_until_ready(oK)
        tK = _time.perf_counter() - t0
        exec_ns = int((tK - t1) / (iters - 1) * 1e9)
    results = [
        {name: np.asarray(out_arrs[i]).reshape(n_cores, *out_avals[i].shape)[c]
         for i, name in enumerate(out_names)}
        for c in range(n_cores)]
    return results, exec_ns


# ----------------------------------------------------------------------------
# entry point
# ----------------------------------------------------------------------------

def kernel(x, edge_attr, pos, params, edge_index):
    x = np.asarray(x, dtype=np.float32)
    edge_attr = np.asarray(edge_attr, dtype=np.float32)
    pos = np.asarray(pos, dtype=np.float32)
    ef_s, rowT, lcolT, invc, xT, x_ownT, T, Em = _host_prep(
        x, edge_attr, pos, edge_index)

    p = {k: np.asarray(v, dtype=np.float32) for k, v in params.items()}
    nc = _build(T, Em)

    common = {"xT": xT}
    for cv, ref in (("c1", "conv1"), ("c2", "conv2")):
        for lay in ("node", "edge", "out"):
            common[f"{cv}_{lay}_w1"] = p[f"{ref}_{lay}_w1"]
            common[f"{cv}_{lay}_b1"] = p[f"{ref}_{lay}_b1"].reshape(D, 1)
            common[f"{cv}_{lay}_w2"] = p[f"{ref}_{lay}_w2"]
            common[f"{cv}_{lay}_b2"] = p[f"{ref}_{lay}_b2"].reshape(1, D)
        common[f"{cv}_bn_g"] = p[f"{ref}_bn_g"].reshape(1, D)
    common["c1_norm_g"] = p["norm1_g"].reshape(1, D)
    common["c1_norm_b"] = p["norm1_b"].reshape(1, D)
    common["c2_norm_g"] = p["norm2_g"].reshape(1, D)
    common["c2_norm_b"] = p["norm2_b"].reshape(1, D)
    for pn in ("q", "k"):
        common[f"{pn}_w"] = p[f"{pn}_w"]
        common[f"{pn}_b"] = p[f"{pn}_b"].reshape(1, D)
    common["v_w"] = p["v_w"]
    common["v_b"] = p["v_b"].reshape(1, D)
    common["out_w"] = p["out_w"]
    common["out_b"] = p["out_b"].reshape(1, D)

    in_maps = []
    for c in range(NCORES):
        m = dict(common)
        m["x_ownT"] = x_ownT[c]
        m["efT"] = ef_s[c]
        m["rowT"] = rowT[c]
        m["lcolT"] = lcolT[c]
        m["invc"] = invc[c]
        in_maps.append({k: np.ascontiguousarray(v, dtype=v.dtype)
                        for k, v in m.items()})

    iters = int(os.environ.get("KERNEL_ITERS", "1"))
    results, exec_ns = _run_pjrt(nc, in_maps, iters=iters)
    LAST_EXEC_TIME_NS[0] = exec_ns
    if int(os.environ.get("KERNEL_DEBUG", "0")):
        kernel.last_debug = results
    out = np.concatenate([results[c]["out"] for c in range(NCORES)], axis=0)
    return out.reshape(16, 512, D).astype(np.float32)


# revision 13
# speedup vs baseline: 24.5288x; 24.5288x over previous
"""Trainium2 Bass kernel for EquivariantUNet block (2x GNN conv + BN + attention).

Sharding: nodes are split into 8 contiguous shards of 1024 (= 2 graphs each).
Edges are bucketed by destination-node 128-block on the host; each core owns
the edges that terminate in its shard. The per-edge gather reads a replicated
node-feature table in DRAM (indirect DMA); scatter-mean is a local one-hot
matmul accumulated in PSUM. Cross-core traffic: one 1MB AllGather (conv2
node-MLP table) + two 2KB AllReduces (global batch-norm statistics).
"""

import os
import numpy as np

import concourse.bass as bass
import concourse.mybir as mybir
import concourse.tile as tile
from concourse import bacc
from concourse.masks import make_identity

F32 = mybir.dt.float32
I32 = mybir.dt.int32
AF = mybir.ActivationFunctionType
OP = mybir.AluOpType

NCORES = 8
N = 8192           # nodes
D = 256            # feature dim
NPC = N // NCORES  # nodes per core (1024)
NBLK = NPC // 128  # 128-node blocks per core (8)
H = 8              # heads
HD = D // H        # head dim (32)
GPC = 2            # graphs per core
EPS_BN = 1e-5
EPS_DIR = 1e-8
EXP_SHIFT = 3.0    # constant softmax shift (mathematically exact)

LAST_EXEC_TIME_NS = [None]


# ----------------------------------------------------------------------------
# host-side preprocessing (sharding metadata + edge features)
# ----------------------------------------------------------------------------

def _sph_harm_np(d):
    x, y, z = d[:, 0], d[:, 1], d[:, 2]
    s3, s5, s15 = 3.0 ** 0.5, 5.0 ** 0.5, 15.0 ** 0.5
    return np.stack([
        np.ones_like(x),
        s3 * x, s3 * y, s3 * z,
        s15 * x * y, s15 * y * z, (s5 / 2.0) * (3.0 * z * z - 1.0),
        s15 * x * z, (s15 / 2.0) * (x * x - y * y),
    ], axis=1).astype(np.float32)


def _host_prep(x, edge_attr, pos, edge_index):
    row = np.asarray(edge_index[0]).astype(np.int64)
    col = np.asarray(edge_index[1]).astype(np.int64)

    rel = pos[row] - pos[col]
    elen = np.sqrt((rel * rel).sum(axis=1, keepdims=True))
    dirs = rel / (elen + EPS_DIR)
    ef_all = np.concatenate([_sph_harm_np(dirs), edge_attr.astype(np.float32)],
                            axis=1)  # [E, 12]

    blk = col // 128  # global destination 128-block, 0..63
    order = np.argsort(blk, kind="stable")
    cnt_blk = np.bincount(blk, minlength=64)
    T = int(np.ceil(cnt_blk.max() / 128))  # tiles per block (uniform, SPMD)
    if (NBLK * T * 128) % 512 != 0:  # always true (1024*T % 512 == 0)
        T += 1
    Em = NBLK * T * 128

    row_s = np.zeros((NCORES, Em), np.int32)
    lcol_s = np.full((NCORES, Em), -1.0, np.float32)
    ef_s = np.zeros((NCORES, 12, Em), np.float32)
    starts = np.zeros(65, np.int64)
    starts[1:] = np.cumsum(cnt_blk)
    for g in range(64):
        c, b = g // NBLK, g % NBLK
        e_ids = order[starts[g]:starts[g + 1]]
        k = e_ids.size
        base = b * T * 128
        row_s[c, base:base + k] = row[e_ids]
        lcol_s[c, base:base + k] = (col[e_ids] - g * 128).astype(np.float32)
        ef_s[c, :, base:base + k] = ef_all[e_ids].T

    rowT = np.ascontiguousarray(
        row_s.reshape(NCORES, Em // 128, 128).transpose(0, 2, 1))
    lcolT = np.ascontiguousarray(
        lcol_s.reshape(NCORES, Em // 128, 128).transpose(0, 2, 1))

    cnt_node = np.bincount(col, minlength=N).astype(np.float32)
    inv_cnt = (1.0 / np.maximum(cnt_node, 1.0)).reshape(NCORES, NBLK, 128)
    invc = np.ascontiguousarray(inv_cnt.transpose(0, 2, 1))

    xT = np.ascontiguousarray(x.T)  # [256, 8192]
    x_ownT = np.ascontiguousarray(
        xT.reshape(D, NCORES, NPC).transpose(1, 0, 2))
    return ef_s, rowT, lcolT, invc, xT, x_ownT, T, Em


# ----------------------------------------------------------------------------
# device kernel
# ----------------------------------------------------------------------------

def _build(T, Em):
    nc = bacc.Bacc(None, target_bir_lowering=False, debug=False,
                   num_devices=NCORES)
    NT = Em // 128
    NCH = Em // 512

    d_xT = nc.dram_tensor("xT", [D, N], F32, kind="ExternalInput")
    d_xoT = nc.dram_tensor("x_ownT", [D, NPC], F32, kind="ExternalInput")
    d_efT = nc.dram_tensor("efT", [12, Em], F32, kind="ExternalInput")
    d_rowT = nc.dram_tensor("rowT", [128, NT], I32, kind="ExternalInput")
    d_lcolT = nc.dram_tensor("lcolT", [128, NT], F32, kind="ExternalInput")
    d_invc = nc.dram_tensor("invc", [128, NBLK], F32, kind="ExternalInput")

    wnames = []
    for cv in ("c1", "c2"):
        wnames += [f"{cv}_node_w1", f"{cv}_node_w2", f"{cv}_edge_w2",
                   f"{cv}_out_w1", f"{cv}_out_w2"]
    wnames += ["q_w", "k_w", "v_w", "out_w"]
    d_w = {}
    for nm in wnames:
        rows = 512 if nm.endswith("out_w1") else 256
        d_w[nm] = nc.dram_tensor(nm, [rows, D], F32, kind="ExternalInput")
    for nm in ("c1_edge_w1", "c2_edge_w1"):
        d_w[nm] = nc.dram_tensor(nm, [12, D], F32, kind="ExternalInput")

    bcol_names, brow_names = [], []
    for cv in ("c1", "c2"):
        bcol_names += [f"{cv}_node_b1", f"{cv}_edge_b1", f"{cv}_out_b1"]
        brow_names += [f"{cv}_node_b2", f"{cv}_edge_b2", f"{cv}_out_b2",
                       f"{cv}_bn_g", f"{cv}_norm_g", f"{cv}_norm_b"]
    brow_names += ["q_b", "k_b", "v_b", "out_b"]
    d_bc = {nm: nc.dram_tensor(nm, [D, 1], F32, kind="ExternalInput")
            for nm in bcol_names}
    d_br = {nm: nc.dram_tensor(nm, [1, D], F32, kind="ExternalInput")
            for nm in brow_names}

    d_out = nc.dram_tensor("out", [NPC, D], F32, kind="ExternalOutput")
    DEBUG = bool(int(os.environ.get("KERNEL_DEBUG", "0")))
    d_dbg = {}
    if DEBUG:
        d_dbg["xt1"] = nc.dram_tensor("dbg_xt1", [NPC, D], F32,
                                      kind="ExternalOutput")
        for nm in ("agg", "h", "agg2", "h2", "y"):
            d_dbg[nm] = nc.dram_tensor(f"dbg_{nm}", [128, NBLK * D], F32,
                                       kind="ExternalOutput")
        d_dbg["stg"] = nc.dram_tensor("dbg_stg", [1, 2 * D], F32,
                                      kind="ExternalOutput")
        d_dbg["ab"] = nc.dram_tensor("dbg_ab", [2, D], F32,
                                     kind="ExternalOutput")

    d_xt1 = nc.dram_tensor("xt1_table", [N, D], F32)
    d_xt2_in = nc.dram_tensor("xt2_bounce", [NPC, D], F32)
    d_xt2 = nc.dram_tensor("xt2_table", [N, D], F32, addr_space="Shared")
    d_st_in = {cv: nc.dram_tensor(f"st_in_{cv}", [1, 2 * D], F32)
               for cv in ("c1", "c2")}
    d_st_out = {cv: nc.dram_tensor(f"st_out_{cv}", [1, 2 * D], F32,
                                   addr_space="Shared")
                for cv in ("c1", "c2")}

    RG = [list(range(NCORES))]

    with tile.TileContext(nc) as tc:
        with tc.tile_pool(name="wpool", bufs=1) as wp, \
             tc.tile_pool(name="persist", bufs=1) as pp:
            # ---- persistent SBUF ------------------------------------------
            w = {}
            for nm in wnames:
                rows = 512 if nm.endswith("out_w1") else 256
                tiles = []
                for k in range(rows // 128):
                    t = wp.tile([128, D], F32, name=f"{nm}_{k}",
                                tag=f"{nm}_{k}")
                    nc.sync.dma_start(out=t[:],
                                      in_=d_w[nm][k * 128:(k + 1) * 128, :])
                    tiles.append(t)
                w[nm] = tiles
            for nm in ("c1_edge_w1", "c2_edge_w1"):
                t = wp.tile([12, D], F32, name=nm, tag=nm)
                nc.sync.dma_start(out=t[:], in_=d_w[nm][:, :])
                w[nm] = t
            bc = {}
            for nm in bcol_names:
                tiles = []
                for k in range(2):
                    t = wp.tile([128, 1], F32, name=f"{nm}_{k}",
                                tag=f"{nm}_{k}")
                    nc.sync.dma_start(out=t[:],
                                      in_=d_bc[nm][k * 128:(k + 1) * 128, :])
                    tiles.append(t)
                bc[nm] = tiles
            br = {}
            for nm in brow_names:
                t = wp.tile([1, D], F32, name=f"br_{nm}", tag=f"br_{nm}")
                nc.sync.dma_start(out=t[:], in_=d_br[nm][:, :])
                br[nm] = t

            ident = wp.tile([128, 128], F32, name="ident", tag="ident")
            make_identity(nc, ident[:])
            iota = wp.tile([128, 128], F32, name="iota", tag="iota")
            nc.gpsimd.iota(iota[:], pattern=[[1, 128]], base=0,
                           channel_multiplier=0,
                           allow_small_or_imprecise_dtypes=True)
            ones_row = wp.tile([1, 512], F32, name="ones_row", tag="ones_row")
            nc.vector.memset(ones_row[:], 1.0)
            ones_col = wp.tile([128, 1], F32, name="ones_col", tag="ones_col")
            nc.vector.memset(ones_col[:], 1.0)
            c_eps = wp.tile([1, 1], F32, name="c_eps", tag="c_eps")
            nc.vector.memset(c_eps[:], EPS_BN)
            c_nsh = wp.tile([128, 1], F32, name="c_nsh", tag="c_nsh")
            nc.vector.memset(c_nsh[:], -EXP_SHIFT)

            t_rowT = wp.tile([128, NT], I32, name="t_rowT", tag="t_rowT")
            nc.sync.dma_start(out=t_rowT[:], in_=d_rowT[:, :])
            t_lcolT = wp.tile([128, NT], F32, name="t_lcolT", tag="t_lcolT")
            nc.sync.dma_start(out=t_lcolT[:], in_=d_lcolT[:, :])
            t_invc = wp.tile([128, NBLK], F32, name="t_invc", tag="t_invc")
            nc.sync.dma_start(out=t_invc[:], in_=d_invc[:, :])

            def ptile(nm, width=NPC):
                return pp.tile([128, width], F32, name=nm, tag=nm)

            h_own = ptile("h_own", NBLK * D)      # conv1 out, node-major
            h_ownT = [ptile(f"h_ownT{i}") for i in range(2)]
            h2_own = ptile("h2_own", NBLK * D)
            h2T = [ptile(f"h2T{i}") for i in range(2)]
            agg_all = ptile("agg_all", NBLK * D)

            # ---------------------------------------------------------------
            def l2_psum(ps, htiles, w2tiles, bias_row, mslice, tag, bufs=2):
                p = ps.tile([128, D], F32, tag=tag, bufs=bufs, name=tag)
                for k in range(len(htiles)):
                    nc.tensor.matmul(p[:], htiles[k][:, mslice], w2tiles[k][:],
                                     start=(k == 0), stop=False)
                nc.tensor.matmul(p[:], ones_row[:, 0:128], bias_row[:],
                                 start=False, stop=True)
                return p

            def transpose_128(ps, dst_slice, src_slice, tag):
                p = ps.tile([128, 128], F32, tag=tag, bufs=2, name=tag)
                nc.tensor.transpose(p[:], src_slice, ident[:])
                nc.vector.tensor_copy(dst_slice, p[:])

            # ===============================================================
            # P1: conv1 node MLP over ALL nodes (replicated) -> xt1_table
            # ===============================================================
            with tc.tile_pool(name="p1ps", bufs=1, space="PSUM") as ps1, \
                 tc.tile_pool(name="p1sb", bufs=1) as sb1:
                for ch in range(N // 512):
                    rhs = []
                    for k in range(2):
                        r = sb1.tile([128, 512], F32, tag="xchunk", bufs=4,
                                     name="xchunk")
                        nc.sync.dma_start(
                            out=r[:], in_=d_xT[k * 128:(k + 1) * 128,
                                              ch * 512:(ch + 1) * 512])
                        rhs.append(r)
                    hs = []
                    for half in range(2):
                        p = ps1.tile([128, 512], F32, tag="n1p", bufs=2,
                                     name="n1p")
                        for ki in range(2):
                            nc.tensor.matmul(
                                p[:],
                                w["c1_node_w1"][ki][:, half * 128:(half + 1) * 128],
                                rhs[ki][:], start=(ki == 0), stop=(ki == 1))
                        o = sb1.tile([128, 512], F32, tag="n1s", bufs=4,
                                     name="n1s")
                        nc.scalar.activation(o[:], p[:], AF.Silu,
                                             bias=bc["c1_node_b1"][half][:, :1])
                        hs.append(o)
                    for i4 in range(4):
                        msl = slice(i4 * 128, (i4 + 1) * 128)
                        p = l2_psum(ps1, hs, w["c1_node_w2"], br["c1_node_b2"],
                                    msl, "n1o")
                        o = sb1.tile([128, D], F32, tag="n1os", bufs=4,
                                     name="n1os")
                        nc.vector.tensor_copy(o[:], p[:])
                        r0 = ch * 512 + i4 * 128
                        nc.sync.dma_start(out=d_xt1[r0:r0 + 128, :], in_=o[:])

            # ===============================================================
            # edge pipeline (both convs)
            # ===============================================================
            agg_state = {}

            def conv_edges(cv, table, dst_tile):
                with tc.tile_pool(name=f"{cv}eps", bufs=1, space="PSUM") as ps, \
                     tc.tile_pool(name=f"{cv}esb", bufs=1) as sb:
                    for ch in range(NCH):
                        ef = sb.tile([12, 512], F32, tag="ef", bufs=4,
                                     name="ef")
                        nc.sync.dma_start(
                            out=ef[:], in_=d_efT[:, ch * 512:(ch + 1) * 512])
                        hs = []
                        for half in range(2):
                            p = ps.tile([128, 512], F32, tag="ehp", bufs=2,
                                        name="ehp")
                            nc.tensor.matmul(
                                p[:],
                                w[f"{cv}_edge_w1"][:, half * 128:(half + 1) * 128],
                                ef[:], start=True, stop=True)
                            o = sb.tile([128, 512], F32, tag="ehs", bufs=4,
                                        name="ehs")
                            nc.scalar.activation(
                                o[:], p[:], AF.Silu,
                                bias=bc[f"{cv}_edge_b1"][half][:, :1])
                            hs.append(o)
                        for i4 in range(4):
                            ti = ch * 4 + i4
                            b, tin = ti // T, ti % T
                            msl = slice(i4 * 128, (i4 + 1) * 128)
                            em_p = l2_psum(ps, hs, w[f"{cv}_edge_w2"],
                                           br[f"{cv}_edge_b2"], msl, "em")
                            xtg = sb.tile([128, D], F32, tag="xtg", bufs=6,
                                          name="xtg")
                            nc.gpsimd.indirect_dma_start(
                                out=xtg[:], out_offset=None, in_=table[:, :],
                                in_offset=bass.IndirectOffsetOnAxis(
                                    ap=t_rowT[:, ti:ti + 1], axis=0))
                            msg = sb.tile([128, D], F32, tag="msg", bufs=4,
                                          name="msg")
                            nc.vector.tensor_tensor(out=msg[:], in0=xtg[:],
                                                    in1=em_p[:], op=OP.mult)
                            oh = sb.tile([128, 128], F32, tag="oh", bufs=4,
                                         name="oh")
                            nc.vector.tensor_scalar(
                                out=oh[:], in0=iota[:],
                                scalar1=t_lcolT[:, ti:ti + 1], scalar2=None,
                                op0=OP.is_equal)
                            if tin == 0:
                                agg_state["p"] = ps.tile(
                                    [128, D], F32, tag="agg", bufs=2,
                                    name="agg")
                            nc.tensor.matmul(agg_state["p"][:], oh[:], msg[:],
                                             start=(tin == 0),
                                             stop=(tin == T - 1))
                            if tin == T - 1:
                                nc.vector.tensor_scalar(
                                    out=dst_tile[:, b * D:(b + 1) * D],
                                    in0=agg_state["p"][:],
                                    scalar1=t_invc[:, b:b + 1], scalar2=None,
                                    op0=OP.mult)

            # ===============================================================
            # out-MLP + fused double-BN + SiLU (both convs)
            # ===============================================================
            def conv_out(cv, inT_tiles, y_dst, yT_dst):
                with tc.tile_pool(name=f"{cv}ops", bufs=1, space="PSUM") as ps, \
                     tc.tile_pool(name=f"{cv}osb", bufs=1) as sb:
                    y_tiles = []
                    st_p = ps.tile([1, 2 * D], F32, tag="st", bufs=1,
                                   name="st_p")
                    for ch2 in range(NPC // 512):
                        hs = []
                        for half in range(2):
                            p = ps.tile([128, 512], F32, tag="o1p", bufs=2,
                                        name="o1p")
                            for ki in range(4):
                                nc.tensor.matmul(
                                    p[:],
                                    w[f"{cv}_out_w1"][ki][:, half * 128:(half + 1) * 128],
                                    inT_tiles[ki][:, ch2 * 512:(ch2 + 1) * 512],
                                    start=(ki == 0), stop=(ki == 3))
                            o = sb.tile([128, 512], F32, tag="o1s", bufs=4,
                                        name="o1s")
                            nc.scalar.activation(
                                o[:], p[:], AF.Silu,
                                bias=bc[f"{cv}_out_b1"][half][:, :1])
                            hs.append(o)
                        for i4 in range(4):
                            bi = ch2 * 4 + i4
                            msl = slice(i4 * 128, (i4 + 1) * 128)
                            p = l2_psum(ps, hs, w[f"{cv}_out_w2"],
                                        br[f"{cv}_out_b2"], msl, "o2")
                            yy = sb.tile([128, 2 * D], F32, tag="yy",
                                         bufs=NBLK + 1, name="yy")
                            nc.vector.tensor_copy(yy[:, 0:D], p[:])
                            nc.vector.tensor_tensor(out=yy[:, D:2 * D],
                                                    in0=yy[:, 0:D],
                                                    in1=yy[:, 0:D],
                                                    op=OP.mult)
                            y_tiles.append(yy)
                            nc.tensor.matmul(st_p[:], ones_col[:], yy[:],
                                             start=(bi == 0),
                                             stop=(bi == NBLK - 1))
                    st_s = sb.tile([1, 2 * D], F32, tag="st_s", name="st_s")
                    nc.vector.tensor_copy(st_s[:], st_p[:])
                    nc.sync.dma_start(out=d_st_in[cv][:, :], in_=st_s[:])
                    nc.gpsimd.collective_compute(
                        "AllReduce", OP.add, replica_groups=RG,
                        ins=[d_st_in[cv].ap()], outs=[d_st_out[cv].ap()])
                    stg = sb.tile([1, 2 * D], F32, tag="stg", name="stg")
                    nc.sync.dma_start(out=stg[:], in_=d_st_out[cv][:, :])

                    def stile(nm):
                        return sb.tile([1, D], F32, tag=nm, name=nm)

                    mu, var, tmp = stile("mu"), stile("var"), stile("tmp")
                    nc.vector.tensor_scalar(out=mu[:], in0=stg[0:1, 0:D],
                                            scalar1=1.0 / N, scalar2=None,
                                            op0=OP.mult)
                    nc.vector.tensor_scalar(out=var[:], in0=stg[0:1, D:2 * D],
                                            scalar1=1.0 / N, scalar2=None,
                                            op0=OP.mult)
                    nc.vector.tensor_tensor(out=tmp[:], in0=mu[:], in1=mu[:],
                                            op=OP.mult)
                    nc.vector.tensor_tensor(out=var[:], in0=var[:], in1=tmp[:],
                                            op=OP.subtract)
                    sq, r1, tt = stile("sq"), stile("r1"), stile("tt")
                    nc.scalar.activation(sq[:], var[:], AF.Sqrt, bias=c_eps[:, :1])
                    nc.vector.reciprocal(r1[:], sq[:])
                    nc.vector.tensor_tensor(out=tt[:], in0=r1[:],
                                            in1=br[f"{cv}_bn_g"][:],
                                            op=OP.mult)
                    vy, sq2, r2 = stile("vy"), stile("sq2"), stile("r2")
                    nc.vector.tensor_tensor(out=vy[:], in0=tt[:], in1=tt[:],
                                            op=OP.mult)
                    nc.vector.tensor_tensor(out=vy[:], in0=vy[:], in1=var[:],
                                            op=OP.mult)
                    nc.scalar.activation(sq2[:], vy[:], AF.Sqrt, bias=c_eps[:, :1])
                    nc.vector.reciprocal(r2[:], sq2[:])
                    Ac, Bc = stile("Ac"), stile("Bc")
                    nc.vector.tensor_tensor(out=Ac[:], in0=tt[:], in1=r2[:],
                                            op=OP.mult)
                    nc.vector.tensor_tensor(out=Ac[:], in0=Ac[:],
                                            in1=br[f"{cv}_norm_g"][:],
                                            op=OP.mult)
                    nc.vector.tensor_tensor(out=Bc[:], in0=mu[:], in1=Ac[:],
                                            op=OP.mult)
                    nc.vector.tensor_tensor(out=Bc[:],
                                            in0=br[f"{cv}_norm_b"][:],
                                            in1=Bc[:], op=OP.subtract)
                    if DEBUG and cv == "c1":
                        nc.sync.dma_start(out=d_dbg["stg"][:, :], in_=stg[:])
                        nc.sync.dma_start(out=d_dbg["ab"][0:1, :], in_=Ac[:])
                        nc.sync.dma_start(out=d_dbg["ab"][1:2, :], in_=Bc[:])
                    # broadcast A/B to 128 partitions via PE
                    Ab = sb.tile([128, D], F32, tag="Ab", name="Ab")
                    Bb = sb.tile([128, D], F32, tag="Bb", name="Bb")
                    for src, dstb in ((Ac, Ab), (Bc, Bb)):
                        pbc = ps.tile([128, D], F32, tag="abp", bufs=1,
                                      name="abp")
                        nc.tensor.matmul(pbc[:], ones_row[:, 0:128], src[:],
                                         start=True, stop=True)
                        nc.vector.tensor_copy(dstb[:], pbc[:])
                    # z = silu(y*A + B)
                    if DEBUG and cv == "c1":
                        for bi in range(NBLK):
                            nc.sync.dma_start(
                                out=d_dbg["y"][:, bi * D:(bi + 1) * D],
                                in_=y_tiles[bi][:, 0:D])
                    for bi in range(NBLK):
                        y = y_tiles[bi]
                        t1 = sb.tile([128, D], F32, tag="zt", bufs=2,
                                     name="zt")
                        nc.vector.tensor_tensor(out=t1[:], in0=y[:, 0:D],
                                                in1=Ab[:], op=OP.mult)
                        nc.vector.tensor_tensor(out=t1[:], in0=t1[:],
                                                in1=Bb[:], op=OP.add)
                        nc.scalar.activation(y_dst[:, bi * D:(bi + 1) * D],
                                             t1[:], AF.Silu)
                        for half in range(2):
                            transpose_128(
                                ps, yT_dst[half][:, bi * 128:(bi + 1) * 128],
                                y_dst[:, bi * D + half * 128:
                                      bi * D + (half + 1) * 128], "ytr")

            # ---- conv1 ----------------------------------------------------
            conv_edges("c1", d_xt1, agg_all)
            if DEBUG:
                nc.sync.dma_start(out=d_dbg["xt1"][:, :], in_=d_xt1[0:NPC, :])
                dbg_agg_sb = pp.tile([128, NBLK * D], F32, name="dbg_agg_sb",
                                     tag="dbg_agg_sb")
                nc.vector.tensor_copy(dbg_agg_sb[:], agg_all[:])
                nc.sync.dma_start(out=d_dbg["agg"][:, :], in_=dbg_agg_sb[:])
            aggT = [ptile(f"aggT{i}") for i in range(2)]
            xoT = [ptile(f"xoT{i}") for i in range(2)]
            with tc.tile_pool(name="c1tps", bufs=1, space="PSUM") as pst:
                for bi in range(NBLK):
                    for half in range(2):
                        transpose_128(
                            pst, aggT[half][:, bi * 128:(bi + 1) * 128],
                            agg_all[:, bi * D + half * 128:
                                    bi * D + (half + 1) * 128], "atr")
            for half in range(2):
                nc.sync.dma_start(out=xoT[half][:],
                                  in_=d_xoT[half * 128:(half + 1) * 128, :])
            conv_out("c1", aggT + xoT, h_own, h_ownT)
            if DEBUG:
                nc.sync.dma_start(out=d_dbg["h"][:, :], in_=h_own[:])

            # ---- conv2 node MLP (own shard) + AllGather --------------------
            with tc.tile_pool(name="p4ps", bufs=1, space="PSUM") as ps4, \
                 tc.tile_pool(name="p4sb", bufs=1) as sb4:
                for ch in range(NPC // 512):
                    hs = []
                    for half in range(2):
                        p = ps4.tile([128, 512], F32, tag="n2p", bufs=2,
                                     name="n2p")
                        for ki in range(2):
                            nc.tensor.matmul(
                                p[:],
                                w["c2_node_w1"][ki][:, half * 128:(half + 1) * 128],
                                h_ownT[ki][:, ch * 512:(ch + 1) * 512],
                                start=(ki == 0), stop=(ki == 1))
                        o = sb4.tile([128, 512], F32, tag="n2s", bufs=4,
                                     name="n2s")
                        nc.scalar.activation(o[:], p[:], AF.Silu,
                                             bias=bc["c2_node_b1"][half][:, :1])
                        hs.append(o)
                    for i4 in range(4):
                        msl = slice(i4 * 128, (i4 + 1) * 128)
                        p = l2_psum(ps4, hs, w["c2_node_w2"], br["c2_node_b2"],
                                    msl, "n2o")
                        o = sb4.tile([128, D], F32, tag="n2os", bufs=4,
                                     name="n2os")
                        nc.vector.tensor_copy(o[:], p[:])
                        r0 = ch * 512 + i4 * 128
                        nc.sync.dma_start(out=d_xt2_in[r0:r0 + 128, :],
                                          in_=o[:])
                nc.gpsimd.collective_compute(
                    "AllGather", OP.bypass, replica_groups=RG,
                    ins=[d_xt2_in.ap()], outs=[d_xt2.ap()])

            # ---- conv2 ----------------------------------------------------
            conv_edges("c2", d_xt2, agg_all)
            if DEBUG:
                nc.sync.dma_start(out=d_dbg["agg2"][:, :], in_=agg_all[:])
            agg2T = [ptile(f"agg2T{i}") for i in range(2)]
            with tc.tile_pool(name="c2tps", bufs=1, space="PSUM") as pst:
                for bi in range(NBLK):
                    for half in range(2):
                        transpose_128(
                            pst, agg2T[half][:, bi * 128:(bi + 1) * 128],
                            agg_all[:, bi * D + half * 128:
                                    bi * D + (half + 1) * 128], "a2tr")
            conv_out("c2", agg2T + h_ownT, h2_own, h2T)
            if DEBUG:
                nc.sync.dma_start(out=d_dbg["h2"][:, :], in_=h2_own[:])

            # ===============================================================
            # attention (2 graphs on own shard)
            # ===============================================================
            OT = [ptile(f"OT{i}") for i in range(2)]
            with tc.tile_pool(name="apsb", bufs=1) as sbp:
              with tc.tile_pool(name="avps", bufs=1, space="PSUM") as psv:
                V = []
                for bi in range(NBLK):
                    msl = slice(bi * 128, (bi + 1) * 128)
                    p = psv.tile([128, D], F32, tag="vp", bufs=2, name="vp")
                    for ki in range(2):
                        nc.tensor.matmul(p[:], h2T[ki][:, msl],
                                         w["v_w"][ki][:],
                                         start=(ki == 0), stop=False)
                    nc.tensor.matmul(p[:], ones_row[:, 0:128], br["v_b"][:],
                                     start=False, stop=True)
                    v = sbp.tile([128, D], F32, tag=f"vsb{bi}", name=f"vsb{bi}")
                    nc.vector.tensor_copy(v[:], p[:])
                    V.append(v)

              with tc.tile_pool(name="ahps", bufs=1, space="PSUM") as psp:
                scale = 1.0 / (HD ** 0.5)
                for g in range(GPC):
                    gsl = slice(g * 512, (g + 1) * 512)
                    for h in range(H):
                        hsl = slice(h * 32, (h + 1) * 32)
                        qk = {}
                        for wn, bn in (("q_w", "q_b"), ("k_w", "k_b")):
                            p = psp.tile([32, 512], F32, tag="qkp", bufs=2,
                                         name="qkp")
                            for ki in range(2):
                                nc.tensor.matmul(p[:], w[wn][ki][:, hsl],
                                                 h2T[ki][:, gsl],
                                                 start=(ki == 0), stop=False)
                            nc.tensor.matmul(p[:], br[bn][:, hsl],
                                             ones_row[:], start=False,
                                             stop=True)
                            t = sbp.tile([32, 512], F32, tag=f"{wn}s", bufs=2,
                                         name=f"{wn}s")
                            nc.scalar.activation(t[:], p[:], AF.Copy)
                            qk[wn] = t
                        exps = []
                        for kt in range(4):
                            s_p = psp.tile([128, 512], F32, tag="sc", bufs=2,
                                           name="s_p")
                            nc.tensor.matmul(
                                s_p[:], qk["k_w"][:, kt * 128:(kt + 1) * 128],
                                qk["q_w"][:], start=True, stop=True)
                            e = sbp.tile([128, 512], F32, tag="exps", bufs=6,
                                         name="exps")
                            nc.scalar.activation(e[:], s_p[:], AF.Exp,
                                                 bias=c_nsh[:, :1],
                                                 scale=scale)
                            exps.append(e)
                        den_p = psp.tile([1, 512], F32, tag="den", bufs=1,
                                         name="den_p")
                        ut_p = psp.tile([32, 512], F32, tag="ut", bufs=2,
                                        name="ut_p")
                        for kt in range(4):
                            nc.tensor.matmul(den_p[:], ones_col[:],
                                             exps[kt][:], start=(kt == 0),
                                             stop=(kt == 3))
                            nc.tensor.matmul(
                                ut_p[:], V[g * 4 + kt][:, hsl],
                                exps[kt][:], start=(kt == 0), stop=(kt == 3))
                        rden = sbp.tile([1, 512], F32, tag="rden", bufs=2,
                                        name="rden")
                        nc.vector.reciprocal(rden[:], den_p[:])
                        rb_p = psp.tile([32, 512], F32, tag="rbp", bufs=1,
                                        name="rb_p")
                        nc.tensor.matmul(rb_p[:], ones_row[:, 0:32], rden[:],
                                         start=True, stop=True)
                        rb = sbp.tile([32, 512], F32, tag="rbs", bufs=2,
                                      name="rb")
                        nc.vector.tensor_copy(rb[:], rb_p[:])
                        ht, hr = h // 4, (h % 4) * 32
                        nc.vector.tensor_tensor(
                            out=OT[ht][hr:hr + 32, gsl],
                            in0=ut_p[:], in1=rb[:], op=OP.mult)

            with tc.tile_pool(name="aops", bufs=1, space="PSUM") as pso, \
                 tc.tile_pool(name="aosb", bufs=1) as sbo:
                for bi in range(NBLK):
                    msl = slice(bi * 128, (bi + 1) * 128)
                    p = pso.tile([128, D], F32, tag="op", bufs=2, name="op")
                    for ki in range(2):
                        nc.tensor.matmul(p[:], OT[ki][:, msl],
                                         w["out_w"][ki][:],
                                         start=(ki == 0), stop=False)
                    nc.tensor.matmul(p[:], ones_row[:, 0:128], br["out_b"][:],
                                     start=False, stop=True)
                    o = sbo.tile([128, D], F32, tag="osb", bufs=4, name="osb")
                    nc.vector.tensor_copy(o[:], p[:])
                    nc.sync.dma_start(out=d_out[bi * 128:(bi + 1) * 128, :],
                                      in_=o[:])

    nc.finalize()
    return nc


# ----------------------------------------------------------------------------
# PJRT runner (axon) with optional repeat-timing
# ----------------------------------------------------------------------------

def _run_pjrt(nc, in_maps, iters=1):
    import time as _time
    import jax
    from jax.experimental.shard_map import shard_map
    from jax.sharding import Mesh, PartitionSpec, NamedSharding
    from concourse.bass2jax import (_bass_exec_p, partition_id_tensor,
                                    install_neuronx_cc_hook)
    import concourse.mybir as mybir_

    install_neuronx_cc_hook()
    n_cores = len(in_maps)
    partition_name = (nc.partition_id_tensor.name
                      if nc.partition_id_tensor else None)
    in_names, out_names, out_avals, zero_outs = [], [], [], []
    for alloc in nc.m.functions[0].allocations:
        if not isinstance(alloc, mybir_.MemoryLocationSet):
            continue
        name = alloc.memorylocations[0].name
        if alloc.kind == "ExternalInput":
            if name != partition_name:
                in_names.append(name)
        elif alloc.kind == "ExternalOutput":
            shape = tuple(alloc.tensor_shape)
            dtype = mybir_.dt.np(alloc.dtype)
            out_names.append(name)
            out_avals.append(jax.core.ShapedArray(shape, dtype))
            zero_outs.append(np.zeros(shape, dtype))
    n_params = len(in_names)
    n_outs = len(out_avals)
    in_names_full = list(in_names) + list(out_names)
    if partition_name is not None:
        in_names_full.append(partition_name)
    donate = tuple(range(n_params, n_params + n_outs))

    def _body(*args):
        operands = list(args)
        if partition_name is not None:
            operands.append(partition_id_tensor())
        outs = _bass_exec_p.bind(
            *operands,
            out_avals=tuple(out_avals),
            in_names=tuple(in_names_full),
            out_names=tuple(out_names),
            lowering_input_output_aliases=(),
            sim_require_finite=True,
            sim_require_nnan=True,
            nc=nc,
        )
        return tuple(outs)

    devices = jax.devices()[:n_cores]
    mesh = Mesh(np.asarray(devices), ("core",))
    in_specs = (PartitionSpec("core"),) * (n_params + n_outs)
    out_specs = (PartitionSpec("core"),) * n_outs
    sharded = jax.jit(
        shard_map(_body, mesh=mesh, in_specs=in_specs, out_specs=out_specs,
                  check_rep=False),
        donate_argnums=donate, keep_unused=True)
    sh = NamedSharding(mesh, PartitionSpec("core"))
    concat_in = [
        jax.device_put(
            np.concatenate([np.asarray(in_maps[c][nm])
                            for c in range(n_cores)], axis=0), sh)
        for nm in in_names]

    def make_zs():
        return [jax.device_put(
            np.zeros((n_cores * z.shape[0], *z.shape[1:]), z.dtype), sh)
            for z in zero_outs]

    out_arrs = sharded(*concat_in, *make_zs())
    jax.block_until_ready(out_arrs)
    exec_ns = None
    if iters > 1:
        zs1 = make_zs()
        zsK = [make_zs() for _ in range(iters)]
        jax.block_until_ready([zs1, zsK])
        t0 = _time.perf_counter()
        o1 = sharded(*concat_in, *zs1)
        jax.block_until_ready(o1)
        t1 = _time.perf_counter() - t0
        t0 = _time.perf_counter()
        oK = [sharded(*concat_in, *zs) for zs in zsK]
        jax.block_until_ready(oK)
        tK = _time.perf_counter() - t0
        exec_ns = int((tK - t1) / (iters - 1) * 1e9)
        print(f"[timing] single-call {t1*1e3:.2f} ms, "
              f"{iters}-call avg {tK/iters*1e3:.2f} ms, "
              f"slope {exec_ns/1e6:.3f} ms", flush=True)
    results = [
        {name: np.asarray(out_arrs[i]).reshape(n_cores, *out_avals[i].shape)[c]
         for i, name in enumerate(out_names)}
        for c in range(n_cores)]
    return results, exec_ns


# ----------------------------------------------------------------------------
# entry point
# ----------------------------------------------------------------------------

def kernel(x, edge_attr, pos, params, edge_index):
    x = np.asarray(x, dtype=np.float32)
    edge_attr = np.asarray(edge_attr, dtype=np.float32)
    pos = np.asarray(pos, dtype=np.float32)
    ef_s, rowT, lcolT, invc, xT, x_ownT, T, Em = _host_prep(
        x, edge_attr, pos, edge_index)

    p = {k: np.asarray(v, dtype=np.float32) for k, v in params.items()}
    nc = _build(T, Em)

    common = {"xT": xT}
    for cv, ref in (("c1", "conv1"), ("c2", "conv2")):
        for lay in ("node", "edge", "out"):
            common[f"{cv}_{lay}_w1"] = p[f"{ref}_{lay}_w1"]
            common[f"{cv}_{lay}_b1"] = p[f"{ref}_{lay}_b1"].reshape(D, 1)
            common[f"{cv}_{lay}_w2"] = p[f"{ref}_{lay}_w2"]
            common[f"{cv}_{lay}_b2"] = p[f"{ref}_{lay}_b2"].reshape(1, D)
        common[f"{cv}_bn_g"] = p[f"{ref}_bn_g"].reshape(1, D)
    common["c1_norm_g"] = p["norm1_g"].reshape(1, D)
    common["c1_norm_b"] = p["norm1_b"].reshape(1, D)
    common["c2_norm_g"] = p["norm2_g"].reshape(1, D)
    common["c2_norm_b"] = p["norm2_b"].reshape(1, D)
    for pn in ("q", "k"):
        common[f"{pn}_w"] = p[f"{pn}_w"]
        common[f"{pn}_b"] = p[f"{pn}_b"].reshape(1, D)
    common["v_w"] = p["v_w"]
    common["v_b"] = p["v_b"].reshape(1, D)
    common["out_w"] = p["out_w"]
    common["out_b"] = p["out_b"].reshape(1, D)

    in_maps = []
    for c in range(NCORES):
        m = dict(common)
        m["x_ownT"] = x_ownT[c]
        m["efT"] = ef_s[c]
        m["rowT"] = rowT[c]
        m["lcolT"] = lcolT[c]
        m["invc"] = invc[c]
        in_maps.append({k: np.ascontiguousarray(v, dtype=v.dtype)
                        for k, v in m.items()})

    iters = int(os.environ.get("KERNEL_ITERS", "1"))
    results, exec_ns = _run_pjrt(nc, in_maps, iters=iters)
    LAST_EXEC_TIME_NS[0] = exec_ns
    if int(os.environ.get("KERNEL_DEBUG", "0")):
        kernel.last_debug = results
    out = np.concatenate([results[c]["out"] for c in range(NCORES)], axis=0)
    return out.reshape(16, 512, D).astype(np.float32)


# revision 17
# speedup vs baseline: 34.3878x; 1.4019x over previous
"""Trainium2 Bass kernel for EquivariantUNet block (2x GNN conv + BN + attention).

Sharding: nodes are split into 8 contiguous shards of 1024 (= 2 graphs each).
Edges are bucketed by destination-node 128-block on the host; each core owns
the edges that terminate in its shard. The per-edge gather reads a replicated
node-feature table in DRAM (indirect DMA); scatter-mean is a local one-hot
matmul accumulated in PSUM. Cross-core traffic: one 1MB AllGather (conv2
node-MLP table) + two 2KB AllReduces (global batch-norm statistics).
"""

import os
import numpy as np

import concourse.bass as bass
import concourse.mybir as mybir
import concourse.tile as tile
from concourse import bacc
from concourse.masks import make_identity

F32 = mybir.dt.float32
I32 = mybir.dt.int32
AF = mybir.ActivationFunctionType
OP = mybir.AluOpType

NCORES = 8
N = 8192           # nodes
D = 256            # feature dim
NPC = N // NCORES  # nodes per core (1024)
NBLK = NPC // 128  # 128-node blocks per core (8)
H = 8              # heads
HD = D // H        # head dim (32)
GPC = 2            # graphs per core
EPS_BN = 1e-5
EPS_DIR = 1e-8
EXP_SHIFT = 3.0    # constant softmax shift (mathematically exact)

LAST_EXEC_TIME_NS = [None]


# ----------------------------------------------------------------------------
# host-side preprocessing (sharding metadata + edge features)
# ----------------------------------------------------------------------------

def _sph_harm_np(d):
    x, y, z = d[:, 0], d[:, 1], d[:, 2]
    s3, s5, s15 = 3.0 ** 0.5, 5.0 ** 0.5, 15.0 ** 0.5
    return np.stack([
        np.ones_like(x),
        s3 * x, s3 * y, s3 * z,
        s15 * x * y, s15 * y * z, (s5 / 2.0) * (3.0 * z * z - 1.0),
        s15 * x * z, (s15 / 2.0) * (x * x - y * y),
    ], axis=1).astype(np.float32)


def _host_prep(x, edge_attr, pos, edge_index):
    row = np.asarray(edge_index[0]).astype(np.int64)
    col = np.asarray(edge_index[1]).astype(np.int64)

    rel = pos[row] - pos[col]
    elen = np.sqrt((rel * rel).sum(axis=1, keepdims=True))
    dirs = rel / (elen + EPS_DIR)
    ef_all = np.concatenate([_sph_harm_np(dirs), edge_attr.astype(np.float32)],
                            axis=1)  # [E, 12]

    blk = col // 128  # global destination 128-block, 0..63
    order = np.argsort(blk, kind="stable")
    cnt_blk = np.bincount(blk, minlength=64)
    T = int(np.ceil(cnt_blk.max() / 128))  # tiles per block (uniform, SPMD)
    if (NBLK * T * 128) % 512 != 0:  # always true (1024*T % 512 == 0)
        T += 1
    Em = NBLK * T * 128

    row_s = np.zeros((NCORES, Em), np.int32)
    lcol_s = np.full((NCORES, Em), -1.0, np.float32)
    ef_s = np.zeros((NCORES, 12, Em), np.float32)
    starts = np.zeros(65, np.int64)
    starts[1:] = np.cumsum(cnt_blk)
    for g in range(64):
        c, b = g // NBLK, g % NBLK
        e_ids = order[starts[g]:starts[g + 1]]
        k = e_ids.size
        base = b * T * 128
        row_s[c, base:base + k] = row[e_ids]
        lcol_s[c, base:base + k] = (col[e_ids] - g * 128).astype(np.float32)
        ef_s[c, :, base:base + k] = ef_all[e_ids].T

    rowT = np.ascontiguousarray(
        row_s.reshape(NCORES, Em // 128, 128).transpose(0, 2, 1))
    lcolT = np.ascontiguousarray(
        lcol_s.reshape(NCORES, Em // 128, 128).transpose(0, 2, 1))

    cnt_node = np.bincount(col, minlength=N).astype(np.float32)
    inv_cnt = (1.0 / np.maximum(cnt_node, 1.0)).reshape(NCORES, NBLK, 128)
    invc = np.ascontiguousarray(inv_cnt.transpose(0, 2, 1))

    xT = np.ascontiguousarray(x.T)  # [256, 8192]
    x_ownT = np.ascontiguousarray(
        xT.reshape(D, NCORES, NPC).transpose(1, 0, 2))
    return ef_s, rowT, lcolT, invc, xT, x_ownT, T, Em


# ----------------------------------------------------------------------------
# device kernel
# ----------------------------------------------------------------------------

def _build(T, Em):
    nc = bacc.Bacc(None, target_bir_lowering=False, debug=False,
                   num_devices=NCORES)
    NT = Em // 128
    NCH = Em // 512

    d_xoT = nc.dram_tensor("x_ownT", [D, NPC], F32, kind="ExternalInput")
    d_efT = nc.dram_tensor("efT", [12, Em], F32, kind="ExternalInput")
    d_rowT = nc.dram_tensor("rowT", [128, NT], I32, kind="ExternalInput")
    d_lcolT = nc.dram_tensor("lcolT", [128, NT], F32, kind="ExternalInput")
    d_invc = nc.dram_tensor("invc", [128, NBLK], F32, kind="ExternalInput")

    wnames = []
    for cv in ("c1", "c2"):
        wnames += [f"{cv}_node_w1", f"{cv}_node_w2", f"{cv}_edge_w2",
                   f"{cv}_out_w1", f"{cv}_out_w2"]
    wnames += ["q_w", "k_w", "v_w", "out_w"]
    d_w = {}
    for nm in wnames:
        rows = 512 if nm.endswith("out_w1") else 256
        d_w[nm] = nc.dram_tensor(nm, [rows, D], F32, kind="ExternalInput")
    for nm in ("c1_edge_w1", "c2_edge_w1"):
        d_w[nm] = nc.dram_tensor(nm, [12, D], F32, kind="ExternalInput")

    bcol_names, brow_names = [], []
    for cv in ("c1", "c2"):
        bcol_names += [f"{cv}_node_b1", f"{cv}_edge_b1", f"{cv}_out_b1"]
        brow_names += [f"{cv}_node_b2", f"{cv}_edge_b2", f"{cv}_out_b2",
                       f"{cv}_bn_g", f"{cv}_norm_g", f"{cv}_norm_b"]
    brow_names += ["q_b", "k_b", "v_b", "out_b"]
    d_bc = {nm: nc.dram_tensor(nm, [D, 1], F32, kind="ExternalInput")
            for nm in bcol_names}
    d_br = {nm: nc.dram_tensor(nm, [1, D], F32, kind="ExternalInput")
            for nm in brow_names}

    d_out = nc.dram_tensor("out", [NPC, D], F32, kind="ExternalOutput")
    DEBUG = bool(int(os.environ.get("KERNEL_DEBUG", "0")))
    d_dbg = {}
    if DEBUG:
        d_dbg["xt1"] = nc.dram_tensor("dbg_xt1", [NPC, D], F32,
                                      kind="ExternalOutput")
        for nm in ("agg", "h", "agg2", "h2", "y"):
            d_dbg[nm] = nc.dram_tensor(f"dbg_{nm}", [128, NBLK * D], F32,
                                       kind="ExternalOutput")
        d_dbg["stg"] = nc.dram_tensor("dbg_stg", [1, 2 * D], F32,
                                      kind="ExternalOutput")
        d_dbg["ab"] = nc.dram_tensor("dbg_ab", [2, D], F32,
                                     kind="ExternalOutput")

    d_xt1_in = nc.dram_tensor("xt1_bounce", [NPC, D], F32)
    d_xt1 = nc.dram_tensor("xt1_table", [N, D], F32, addr_space="Shared")
    d_xt2_in = nc.dram_tensor("xt2_bounce", [NPC, D], F32)
    d_xt2 = nc.dram_tensor("xt2_table", [N, D], F32, addr_space="Shared")
    d_st_in = {cv: nc.dram_tensor(f"st_in_{cv}", [1, 2 * D], F32)
               for cv in ("c1", "c2")}
    d_st_out = {cv: nc.dram_tensor(f"st_out_{cv}", [1, 2 * D], F32,
                                   addr_space="Shared")
                for cv in ("c1", "c2")}

    RG = [list(range(NCORES))]
    NOCC = bool(int(os.environ.get("KERNEL_NOCC", "0")))

    with tile.TileContext(nc) as tc:
        with tc.tile_pool(name="wpool", bufs=1) as wp, \
             tc.tile_pool(name="persist", bufs=1) as pp:
            # ---- persistent SBUF ------------------------------------------
            w = {}
            for nm in wnames:
                rows = 512 if nm.endswith("out_w1") else 256
                tiles = []
                for k in range(rows // 128):
                    t = wp.tile([128, D], F32, name=f"{nm}_{k}",
                                tag=f"{nm}_{k}")
                    nc.sync.dma_start(out=t[:],
                                      in_=d_w[nm][k * 128:(k + 1) * 128, :])
                    tiles.append(t)
                w[nm] = tiles
            for nm in ("c1_edge_w1", "c2_edge_w1"):
                t = wp.tile([12, D], F32, name=nm, tag=nm)
                nc.sync.dma_start(out=t[:], in_=d_w[nm][:, :])
                w[nm] = t
            bc = {}
            for nm in bcol_names:
                tiles = []
                for k in range(2):
                    t = wp.tile([128, 1], F32, name=f"{nm}_{k}",
                                tag=f"{nm}_{k}")
                    nc.sync.dma_start(out=t[:],
                                      in_=d_bc[nm][k * 128:(k + 1) * 128, :])
                    tiles.append(t)
                bc[nm] = tiles
            br = {}
            for nm in brow_names:
                t = wp.tile([1, D], F32, name=f"br_{nm}", tag=f"br_{nm}")
                nc.sync.dma_start(out=t[:], in_=d_br[nm][:, :])
                br[nm] = t

            ident = wp.tile([128, 128], F32, name="ident", tag="ident")
            make_identity(nc, ident[:])
            iota = wp.tile([128, 128], F32, name="iota", tag="iota")
            nc.gpsimd.iota(iota[:], pattern=[[1, 128]], base=0,
                           channel_multiplier=0,
                           allow_small_or_imprecise_dtypes=True)
            ones_row = wp.tile([1, 512], F32, name="ones_row", tag="ones_row")
            nc.vector.memset(ones_row[:], 1.0)
            ones_col = wp.tile([128, 1], F32, name="ones_col", tag="ones_col")
            nc.vector.memset(ones_col[:], 1.0)
            c_eps = wp.tile([1, 1], F32, name="c_eps", tag="c_eps")
            nc.vector.memset(c_eps[:], EPS_BN)
            c_nsh = wp.tile([128, 1], F32, name="c_nsh", tag="c_nsh")
            nc.vector.memset(c_nsh[:], -EXP_SHIFT)

            t_rowT = wp.tile([128, NT], I32, name="t_rowT", tag="t_rowT")
            nc.sync.dma_start(out=t_rowT[:], in_=d_rowT[:, :])
            t_lcolT = wp.tile([128, NT], F32, name="t_lcolT", tag="t_lcolT")
            nc.sync.dma_start(out=t_lcolT[:], in_=d_lcolT[:, :])
            t_invc = wp.tile([128, NBLK], F32, name="t_invc", tag="t_invc")
            nc.sync.dma_start(out=t_invc[:], in_=d_invc[:, :])

            # broadcast row-biases to [128, D] once (PE ones-outer-product)
            bb = {}
            with tc.tile_pool(name="bbps", bufs=2, space="PSUM") as psb:
                for nm in ("c1_node_b2", "c2_node_b2", "c1_edge_b2",
                           "c2_edge_b2", "c1_out_b2", "c2_out_b2",
                           "v_b", "out_b"):
                    pbt = psb.tile([128, D], F32, tag="bbp", bufs=2,
                                   name="bbp")
                    nc.tensor.matmul(pbt[:], ones_row[:, 0:128], br[nm][:],
                                     start=True, stop=True)
                    t = wp.tile([128, D], F32, name=f"bb_{nm}", tag=f"bb_{nm}")
                    nc.vector.tensor_copy(t[:], pbt[:])
                    bb[nm] = t

            def ptile(nm, width=NPC):
                return pp.tile([128, width], F32, name=nm, tag=nm)

            h_own = ptile("h_own", NBLK * D)      # conv1 out, node-major
            h_ownT = [ptile(f"h_ownT{i}") for i in range(2)]
            h2_own = ptile("h2_own", NBLK * D)
            h2T = [ptile(f"h2T{i}") for i in range(2)]
            agg_all = ptile("agg_all", NBLK * D)

            # ---------------------------------------------------------------
            def l2_psum(ps, htiles, w2tiles, mslice, tag, bufs=2):
                p = ps.tile([128, D], F32, tag=tag, bufs=bufs, name=tag)
                nk = len(htiles)
                for k in range(nk):
                    nc.tensor.matmul(p[:], htiles[k][:, mslice], w2tiles[k][:],
                                     start=(k == 0), stop=(k == nk - 1))
                return p

            def transpose_128(ps, dst_slice, src_slice, tag):
                p = ps.tile([128, 128], F32, tag=tag, bufs=2, name=tag)
                nc.tensor.transpose(p[:], src_slice, ident[:])
                nc.vector.tensor_copy(dst_slice, p[:])

            # ===============================================================
            # P1: conv1 node MLP (own shard) + AllGather -> xt1_table
            # ===============================================================
            xoT = [ptile(f"xoT{i}") for i in range(2)]
            for half in range(2):
                nc.sync.dma_start(out=xoT[half][:],
                                  in_=d_xoT[half * 128:(half + 1) * 128, :])
            with tc.tile_pool(name="p1ps", bufs=1, space="PSUM") as ps1, \
                 tc.tile_pool(name="p1sb", bufs=1) as sb1:
                for ch in range(NPC // 512):
                    hs = []
                    for half in range(2):
                        p = ps1.tile([128, 512], F32, tag="n1p", bufs=2,
                                     name="n1p")
                        for ki in range(2):
                            nc.tensor.matmul(
                                p[:],
                                w["c1_node_w1"][ki][:, half * 128:(half + 1) * 128],
                                xoT[ki][:, ch * 512:(ch + 1) * 512],
                                start=(ki == 0), stop=(ki == 1))
                        o = sb1.tile([128, 512], F32, tag="n1s", bufs=4,
                                     name="n1s")
                        nc.scalar.activation(o[:], p[:], AF.Silu,
                                             bias=bc["c1_node_b1"][half][:, :1])
                        hs.append(o)
                    for i4 in range(4):
                        msl = slice(i4 * 128, (i4 + 1) * 128)
                        p = l2_psum(ps1, hs, w["c1_node_w2"], msl, "n1o")
                        o = sb1.tile([128, D], F32, tag="n1os", bufs=4,
                                     name="n1os")
                        nc.vector.tensor_tensor(out=o[:], in0=p[:],
                                                in1=bb["c1_node_b2"][:],
                                                op=OP.add)
                        r0 = ch * 512 + i4 * 128
                        nc.sync.dma_start(out=d_xt1_in[r0:r0 + 128, :],
                                          in_=o[:])
                if not NOCC:
                    nc.gpsimd.collective_compute(
                        "AllGather", OP.bypass, replica_groups=RG,
                        ins=[d_xt1_in.ap()], outs=[d_xt1.ap()])
                else:
                    nc.sync.dma_start(out=d_xt1[0:NPC, :], in_=d_xt1_in[:, :])

            # ===============================================================
            # edge pipeline (both convs)
            # ===============================================================
            agg_state = {}

            def conv_edges(cv, table, dst_tile):
                with tc.tile_pool(name=f"{cv}eps", bufs=1, space="PSUM") as ps, \
                     tc.tile_pool(name=f"{cv}esb", bufs=1) as sb:
                    for ch in range(NCH):
                        ef = sb.tile([12, 512], F32, tag="ef", bufs=4,
                                     name="ef")
                        nc.sync.dma_start(
                            out=ef[:], in_=d_efT[:, ch * 512:(ch + 1) * 512])
                        hs = []
                        for half in range(2):
                            p = ps.tile([128, 512], F32, tag="ehp", bufs=2,
                                        name="ehp")
                            nc.tensor.matmul(
                                p[:],
                                w[f"{cv}_edge_w1"][:, half * 128:(half + 1) * 128],
                                ef[:], start=True, stop=True)
                            o = sb.tile([128, 512], F32, tag="ehs", bufs=4,
                                        name="ehs")
                            nc.scalar.activation(
                                o[:], p[:], AF.Silu,
                                bias=bc[f"{cv}_edge_b1"][half][:, :1])
                            hs.append(o)
                        for i4 in range(4):
                            ti = ch * 4 + i4
                            b, tin = ti // T, ti % T
                            msl = slice(i4 * 128, (i4 + 1) * 128)
                            em_p = l2_psum(ps, hs, w[f"{cv}_edge_w2"],
                                           msl, "em")
                            xtg = sb.tile([128, D], F32, tag="xtg", bufs=6,
                                          name="xtg")
                            nc.gpsimd.indirect_dma_start(
                                out=xtg[:], out_offset=None, in_=table[:, :],
                                in_offset=bass.IndirectOffsetOnAxis(
                                    ap=t_rowT[:, ti:ti + 1], axis=0))
                            msg = sb.tile([128, D], F32, tag="msg", bufs=4,
                                          name="msg")
                            nc.vector.tensor_tensor(
                                out=msg[:], in0=em_p[:],
                                in1=bb[f"{cv}_edge_b2"][:], op=OP.add)
                            nc.vector.tensor_tensor(out=msg[:], in0=msg[:],
                                                    in1=xtg[:], op=OP.mult)
                            oh = sb.tile([128, 128], F32, tag="oh", bufs=4,
                                         name="oh")
                            nc.vector.tensor_scalar(
                                out=oh[:], in0=iota[:],
                                scalar1=t_lcolT[:, ti:ti + 1], scalar2=None,
                                op0=OP.is_equal)
                            if tin == 0:
                                agg_state["p"] = ps.tile(
                                    [128, D], F32, tag="agg", bufs=2,
                                    name="agg")
                            nc.tensor.matmul(agg_state["p"][:], oh[:], msg[:],
                                             start=(tin == 0),
                                             stop=(tin == T - 1))
                            if tin == T - 1:
                                nc.vector.tensor_scalar(
                                    out=dst_tile[:, b * D:(b + 1) * D],
                                    in0=agg_state["p"][:],
                                    scalar1=t_invc[:, b:b + 1], scalar2=None,
                                    op0=OP.mult)

            # ===============================================================
            # out-MLP + fused double-BN + SiLU (both convs)
            # ===============================================================
            def conv_out(cv, inT_tiles, y_dst, yT_dst):
                with tc.tile_pool(name=f"{cv}ops", bufs=1, space="PSUM") as ps, \
                     tc.tile_pool(name=f"{cv}osb", bufs=1) as sb:
                    y_tiles = []
                    st_p = ps.tile([1, 2 * D], F32, tag="st", bufs=1,
                                   name="st_p")
                    for ch2 in range(NPC // 512):
                        hs = []
                        for half in range(2):
                            p = ps.tile([128, 512], F32, tag="o1p", bufs=2,
                                        name="o1p")
                            for ki in range(4):
                                nc.tensor.matmul(
                                    p[:],
                                    w[f"{cv}_out_w1"][ki][:, half * 128:(half + 1) * 128],
                                    inT_tiles[ki][:, ch2 * 512:(ch2 + 1) * 512],
                                    start=(ki == 0), stop=(ki == 3))
                            o = sb.tile([128, 512], F32, tag="o1s", bufs=4,
                                        name="o1s")
                            nc.scalar.activation(
                                o[:], p[:], AF.Silu,
                                bias=bc[f"{cv}_out_b1"][half][:, :1])
                            hs.append(o)
                        for i4 in range(4):
                            bi = ch2 * 4 + i4
                            msl = slice(i4 * 128, (i4 + 1) * 128)
                            p = l2_psum(ps, hs, w[f"{cv}_out_w2"],
                                        msl, "o2")
                            yy = sb.tile([128, 2 * D], F32, tag="yy",
                                         bufs=NBLK + 1, name="yy")
                            nc.vector.tensor_tensor(
                                out=yy[:, 0:D], in0=p[:],
                                in1=bb[f"{cv}_out_b2"][:], op=OP.add)
                            nc.vector.tensor_tensor(out=yy[:, D:2 * D],
                                                    in0=yy[:, 0:D],
                                                    in1=yy[:, 0:D],
                                                    op=OP.mult)
                            y_tiles.append(yy)
                            nc.tensor.matmul(st_p[:], ones_col[:], yy[:],
                                             start=(bi == 0),
                                             stop=(bi == NBLK - 1))
                    st_s = sb.tile([1, 2 * D], F32, tag="st_s", name="st_s")
                    nc.vector.tensor_copy(st_s[:], st_p[:])
                    nc.sync.dma_start(out=d_st_in[cv][:, :], in_=st_s[:])
                    if not NOCC:
                        nc.gpsimd.collective_compute(
                            "AllReduce", OP.add, replica_groups=RG,
                            ins=[d_st_in[cv].ap()], outs=[d_st_out[cv].ap()])
                    stg = sb.tile([1, 2 * D], F32, tag="stg", name="stg")
                    nc.sync.dma_start(out=stg[:], in_=d_st_out[cv][:, :]
                                      if not NOCC else d_st_in[cv][:, :])

                    def stile(nm):
                        return sb.tile([1, D], F32, tag=nm, name=nm)

                    mu, var, tmp = stile("mu"), stile("var"), stile("tmp")
                    nc.vector.tensor_scalar(out=mu[:], in0=stg[0:1, 0:D],
                                            scalar1=1.0 / N, scalar2=None,
                                            op0=OP.mult)
                    nc.vector.tensor_scalar(out=var[:], in0=stg[0:1, D:2 * D],
                                            scalar1=1.0 / N, scalar2=None,
                                            op0=OP.mult)
                    nc.vector.tensor_tensor(out=tmp[:], in0=mu[:], in1=mu[:],
                                            op=OP.mult)
                    nc.vector.tensor_tensor(out=var[:], in0=var[:], in1=tmp[:],
                                            op=OP.subtract)
                    sq, r1, tt = stile("sq"), stile("r1"), stile("tt")
                    nc.scalar.activation(sq[:], var[:], AF.Sqrt, bias=c_eps[:, :1])
                    nc.vector.reciprocal(r1[:], sq[:])
                    nc.vector.tensor_tensor(out=tt[:], in0=r1[:],
                                            in1=br[f"{cv}_bn_g"][:],
                                            op=OP.mult)
                    vy, sq2, r2 = stile("vy"), stile("sq2"), stile("r2")
                    nc.vector.tensor_tensor(out=vy[:], in0=tt[:], in1=tt[:],
                                            op=OP.mult)
                    nc.vector.tensor_tensor(out=vy[:], in0=vy[:], in1=var[:],
                                            op=OP.mult)
                    nc.scalar.activation(sq2[:], vy[:], AF.Sqrt, bias=c_eps[:, :1])
                    nc.vector.reciprocal(r2[:], sq2[:])
                    Ac, Bc = stile("Ac"), stile("Bc")
                    nc.vector.tensor_tensor(out=Ac[:], in0=tt[:], in1=r2[:],
                                            op=OP.mult)
                    nc.vector.tensor_tensor(out=Ac[:], in0=Ac[:],
                                            in1=br[f"{cv}_norm_g"][:],
                                            op=OP.mult)
                    nc.vector.tensor_tensor(out=Bc[:], in0=mu[:], in1=Ac[:],
                                            op=OP.mult)
                    nc.vector.tensor_tensor(out=Bc[:],
                                            in0=br[f"{cv}_norm_b"][:],
                                            in1=Bc[:], op=OP.subtract)
                    if DEBUG and cv == "c1":
                        nc.sync.dma_start(out=d_dbg["stg"][:, :], in_=stg[:])
                        nc.sync.dma_start(out=d_dbg["ab"][0:1, :], in_=Ac[:])
                        nc.sync.dma_start(out=d_dbg["ab"][1:2, :], in_=Bc[:])
                    # broadcast A/B to 128 partitions via PE
                    Ab = sb.tile([128, D], F32, tag="Ab", name="Ab")
                    Bb = sb.tile([128, D], F32, tag="Bb", name="Bb")
                    for src, dstb in ((Ac, Ab), (Bc, Bb)):
                        pbc = ps.tile([128, D], F32, tag="abp", bufs=1,
                                      name="abp")
                        nc.tensor.matmul(pbc[:], ones_row[:, 0:128], src[:],
                                         start=True, stop=True)
                        nc.vector.tensor_copy(dstb[:], pbc[:])
                    # z = silu(y*A + B)
                    if DEBUG and cv == "c1":
                        for bi in range(NBLK):
                            nc.sync.dma_start(
                                out=d_dbg["y"][:, bi * D:(bi + 1) * D],
                                in_=y_tiles[bi][:, 0:D])
                    for bi in range(NBLK):
                        y = y_tiles[bi]
                        t1 = sb.tile([128, D], F32, tag="zt", bufs=2,
                                     name="zt")
                        nc.vector.tensor_tensor(out=t1[:], in0=y[:, 0:D],
                                                in1=Ab[:], op=OP.mult)
                        nc.vector.tensor_tensor(out=t1[:], in0=t1[:],
                                                in1=Bb[:], op=OP.add)
                        nc.scalar.activation(y_dst[:, bi * D:(bi + 1) * D],
                                             t1[:], AF.Silu)
                        for half in range(2):
                            transpose_128(
                                ps, yT_dst[half][:, bi * 128:(bi + 1) * 128],
                                y_dst[:, bi * D + half * 128:
                                      bi * D + (half + 1) * 128], "ytr")

            # ---- conv1 ----------------------------------------------------
            conv_edges("c1", d_xt1, agg_all)
            if DEBUG:
                nc.sync.dma_start(out=d_dbg["xt1"][:, :], in_=d_xt1[0:NPC, :])
                dbg_agg_sb = pp.tile([128, NBLK * D], F32, name="dbg_agg_sb",
                                     tag="dbg_agg_sb")
                nc.vector.tensor_copy(dbg_agg_sb[:], agg_all[:])
                nc.sync.dma_start(out=d_dbg["agg"][:, :], in_=dbg_agg_sb[:])
            aggT = [ptile(f"aggT{i}") for i in range(2)]
            with tc.tile_pool(name="c1tps", bufs=1, space="PSUM") as pst:
                for bi in range(NBLK):
                    for half in range(2):
                        transpose_128(
                            pst, aggT[half][:, bi * 128:(bi + 1) * 128],
                            agg_all[:, bi * D + half * 128:
                                    bi * D + (half + 1) * 128], "atr")
            conv_out("c1", aggT + xoT, h_own, h_ownT)
            if DEBUG:
                nc.sync.dma_start(out=d_dbg["h"][:, :], in_=h_own[:])

            # ---- conv2 node MLP (own shard) + AllGather --------------------
            with tc.tile_pool(name="p4ps", bufs=1, space="PSUM") as ps4, \
                 tc.tile_pool(name="p4sb", bufs=1) as sb4:
                for ch in range(NPC // 512):
                    hs = []
                    for half in range(2):
                        p = ps4.tile([128, 512], F32, tag="n2p", bufs=2,
                                     name="n2p")
                        for ki in range(2):
                            nc.tensor.matmul(
                                p[:],
                                w["c2_node_w1"][ki][:, half * 128:(half + 1) * 128],
                                h_ownT[ki][:, ch * 512:(ch + 1) * 512],
                                start=(ki == 0), stop=(ki == 1))
                        o = sb4.tile([128, 512], F32, tag="n2s", bufs=4,
                                     name="n2s")
                        nc.scalar.activation(o[:], p[:], AF.Silu,
                                             bias=bc["c2_node_b1"][half][:, :1])
                        hs.append(o)
                    for i4 in range(4):
                        msl = slice(i4 * 128, (i4 + 1) * 128)
                        p = l2_psum(ps4, hs, w["c2_node_w2"], msl, "n2o")
                        o = sb4.tile([128, D], F32, tag="n2os", bufs=4,
                                     name="n2os")
                        nc.vector.tensor_tensor(out=o[:], in0=p[:],
                                                in1=bb["c2_node_b2"][:],
                                                op=OP.add)
                        r0 = ch * 512 + i4 * 128
                        nc.sync.dma_start(out=d_xt2_in[r0:r0 + 128, :],
                                          in_=o[:])
                if not NOCC:
                    nc.gpsimd.collective_compute(
                        "AllGather", OP.bypass, replica_groups=RG,
                        ins=[d_xt2_in.ap()], outs=[d_xt2.ap()])
                else:
                    nc.sync.dma_start(out=d_xt2[0:NPC, :], in_=d_xt2_in[:, :])

            # ---- conv2 ----------------------------------------------------
            conv_edges("c2", d_xt2, agg_all)
            if DEBUG:
                nc.sync.dma_start(out=d_dbg["agg2"][:, :], in_=agg_all[:])
            agg2T = [ptile(f"agg2T{i}") for i in range(2)]
            with tc.tile_pool(name="c2tps", bufs=1, space="PSUM") as pst:
                for bi in range(NBLK):
                    for half in range(2):
                        transpose_128(
                            pst, agg2T[half][:, bi * 128:(bi + 1) * 128],
                            agg_all[:, bi * D + half * 128:
                                    bi * D + (half + 1) * 128], "a2tr")
            conv_out("c2", agg2T + h_ownT, h2_own, h2T)
            if DEBUG:
                nc.sync.dma_start(out=d_dbg["h2"][:, :], in_=h2_own[:])

            # ===============================================================
            # attention (2 graphs on own shard)
            # ===============================================================
            OT = [ptile(f"OT{i}") for i in range(2)]
            with tc.tile_pool(name="apsb", bufs=1) as sbp:
              with tc.tile_pool(name="avps", bufs=1, space="PSUM") as psv:
                V = []
                for bi in range(NBLK):
                    msl = slice(bi * 128, (bi + 1) * 128)
                    p = psv.tile([128, D], F32, tag="vp", bufs=2, name="vp")
                    for ki in range(2):
                        nc.tensor.matmul(p[:], h2T[ki][:, msl],
                                         w["v_w"][ki][:],
                                         start=(ki == 0), stop=(ki == 1))
                    v = sbp.tile([128, D], F32, tag=f"vsb{bi}", name=f"vsb{bi}")
                    nc.vector.tensor_tensor(out=v[:], in0=p[:],
                                            in1=bb["v_b"][:], op=OP.add)
                    V.append(v)

              with tc.tile_pool(name="ahps", bufs=1, space="PSUM") as psp:
                scale = 1.0 / (HD ** 0.5)
                for g in range(GPC):
                    gsl = slice(g * 512, (g + 1) * 512)
                    for h in range(H):
                        hsl = slice(h * 32, (h + 1) * 32)
                        qk = {}
                        for wn, bn in (("q_w", "q_b"), ("k_w", "k_b")):
                            p = psp.tile([32, 512], F32, tag="qkp", bufs=2,
                                         name="qkp")
                            for ki in range(2):
                                nc.tensor.matmul(p[:], w[wn][ki][:, hsl],
                                                 h2T[ki][:, gsl],
                                                 start=(ki == 0), stop=False)
                            nc.tensor.matmul(p[:], br[bn][:, hsl],
                                             ones_row[:], start=False,
                                             stop=True)
                            t = sbp.tile([32, 512], F32, tag=f"{wn}s", bufs=2,
                                         name=f"{wn}s")
                            nc.scalar.activation(t[:], p[:], AF.Copy)
                            qk[wn] = t
                        exps = []
                        for kt in range(4):
                            s_p = psp.tile([128, 512], F32, tag="sc", bufs=2,
                                           name="s_p")
                            nc.tensor.matmul(
                                s_p[:], qk["k_w"][:, kt * 128:(kt + 1) * 128],
                                qk["q_w"][:], start=True, stop=True)
                            e = sbp.tile([128, 512], F32, tag="exps", bufs=6,
                                         name="exps")
                            nc.scalar.activation(e[:], s_p[:], AF.Exp,
                                                 bias=c_nsh[:, :1],
                                                 scale=scale)
                            exps.append(e)
                        den_p = psp.tile([1, 512], F32, tag="den", bufs=1,
                                         name="den_p")
                        ut_p = psp.tile([32, 512], F32, tag="ut", bufs=2,
                                        name="ut_p")
                        for kt in range(4):
                            nc.tensor.matmul(den_p[:], ones_col[:],
                                             exps[kt][:], start=(kt == 0),
                                             stop=(kt == 3))
                            nc.tensor.matmul(
                                ut_p[:], V[g * 4 + kt][:, hsl],
                                exps[kt][:], start=(kt == 0), stop=(kt == 3))
                        rden = sbp.tile([1, 512], F32, tag="rden", bufs=2,
                                        name="rden")
                        nc.vector.reciprocal(rden[:], den_p[:])
                        rb_p = psp.tile([32, 512], F32, tag="rbp", bufs=1,
                                        name="rb_p")
                        nc.tensor.matmul(rb_p[:], ones_row[:, 0:32], rden[:],
                                         start=True, stop=True)
                        rb = sbp.tile([32, 512], F32, tag="rbs", bufs=2,
                                      name="rb")
                        nc.vector.tensor_copy(rb[:], rb_p[:])
                        ht, hr = h // 4, (h % 4) * 32
                        nc.vector.tensor_tensor(
                            out=OT[ht][hr:hr + 32, gsl],
                            in0=ut_p[:], in1=rb[:], op=OP.mult)

            with tc.tile_pool(name="aops", bufs=1, space="PSUM") as pso, \
                 tc.tile_pool(name="aosb", bufs=1) as sbo:
                for bi in range(NBLK):
                    msl = slice(bi * 128, (bi + 1) * 128)
                    p = pso.tile([128, D], F32, tag="op", bufs=2, name="op")
                    for ki in range(2):
                        nc.tensor.matmul(p[:], OT[ki][:, msl],
                                         w["out_w"][ki][:],
                                         start=(ki == 0), stop=(ki == 1))
                    o = sbo.tile([128, D], F32, tag="osb", bufs=4, name="osb")
                    nc.vector.tensor_tensor(out=o[:], in0=p[:],
                                            in1=bb["out_b"][:], op=OP.add)
                    nc.sync.dma_start(out=d_out[bi * 128:(bi + 1) * 128, :],
                                      in_=o[:])

    nc.finalize()
    return nc


# ----------------------------------------------------------------------------
# PJRT runner (axon) with optional repeat-timing
# ----------------------------------------------------------------------------

def _run_pjrt(nc, in_maps, iters=1):
    import time as _time
    import jax
    from jax.experimental.shard_map import shard_map
    from jax.sharding import Mesh, PartitionSpec, NamedSharding
    from concourse.bass2jax import (_bass_exec_p, partition_id_tensor,
                                    install_neuronx_cc_hook)
    import concourse.mybir as mybir_

    install_neuronx_cc_hook()
    n_cores = len(in_maps)
    partition_name = (nc.partition_id_tensor.name
                      if nc.partition_id_tensor else None)
    in_names, out_names, out_avals, zero_outs = [], [], [], []
    for alloc in nc.m.functions[0].allocations:
        if not isinstance(alloc, mybir_.MemoryLocationSet):
            continue
        name = alloc.memorylocations[0].name
        if alloc.kind == "ExternalInput":
            if name != partition_name:
                in_names.append(name)
        elif alloc.kind == "ExternalOutput":
            shape = tuple(alloc.tensor_shape)
            dtype = mybir_.dt.np(alloc.dtype)
            out_names.append(name)
            out_avals.append(jax.core.ShapedArray(shape, dtype))
            zero_outs.append(np.zeros(shape, dtype))
    n_params = len(in_names)
    n_outs = len(out_avals)
    in_names_full = list(in_names) + list(out_names)
    if partition_name is not None:
        in_names_full.append(partition_name)
    donate = tuple(range(n_params, n_params + n_outs))

    def _body(*args):
        operands = list(args)
        if partition_name is not None:
            operands.append(partition_id_tensor())
        outs = _bass_exec_p.bind(
            *operands,
            out_avals=tuple(out_avals),
            in_names=tuple(in_names_full),
            out_names=tuple(out_names),
            lowering_input_output_aliases=(),
            sim_require_finite=True,
            sim_require_nnan=True,
            nc=nc,
        )
        return tuple(outs)

    devices = jax.devices()[:n_cores]
    mesh = Mesh(np.asarray(devices), ("core",))
    in_specs = (PartitionSpec("core"),) * (n_params + n_outs)
    out_specs = (PartitionSpec("core"),) * n_outs
    sharded = jax.jit(
        shard_map(_body, mesh=mesh, in_specs=in_specs, out_specs=out_specs,
                  check_rep=False),
        donate_argnums=donate, keep_unused=True)
    sh = NamedSharding(mesh, PartitionSpec("core"))
    concat_in = [
        jax.device_put(
            np.concatenate([np.asarray(in_maps[c][nm])
                            for c in range(n_cores)], axis=0), sh)
        for nm in in_names]

    def make_zs():
        return [jax.device_put(
            np.zeros((n_cores * z.shape[0], *z.shape[1:]), z.dtype), sh)
            for z in zero_outs]

    out_arrs = sharded(*concat_in, *make_zs())
    jax.block_until_ready(out_arrs)
    exec_ns = None
    if iters > 1:
        def timed_batch(k):
            zsl = [make_zs() for _ in range(k)]
            jax.block_until_ready(zsl)
            t0 = _time.perf_counter()
            outs = [sharded(*concat_in, *zs) for zs in zsl]
            jax.block_until_ready(outs)
            return _time.perf_counter() - t0
        timed_batch(2)  # warm
        t4 = timed_batch(4)
        t16 = timed_batch(16)
        exec_ns = int((t16 - t4) / 12 * 1e9)
        print(f"[timing] 4-call avg {t4/4*1e3:.2f} ms, "
              f"16-call avg {t16/16*1e3:.2f} ms, "
              f"slope {exec_ns/1e6:.3f} ms", flush=True)
    results = [
        {name: np.asarray(out_arrs[i]).reshape(n_cores, *out_avals[i].shape)[c]
         for i, name in enumerate(out_names)}
        for c in range(n_cores)]
    return results, exec_ns


# ----------------------------------------------------------------------------
# entry point
# ----------------------------------------------------------------------------

def kernel(x, edge_attr, pos, params, edge_index):
    x = np.asarray(x, dtype=np.float32)
    edge_attr = np.asarray(edge_attr, dtype=np.float32)
    pos = np.asarray(pos, dtype=np.float32)
    ef_s, rowT, lcolT, invc, xT, x_ownT, T, Em = _host_prep(
        x, edge_attr, pos, edge_index)

    p = {k: np.asarray(v, dtype=np.float32) for k, v in params.items()}
    nc = _build(T, Em)

    common = {}
    for cv, ref in (("c1", "conv1"), ("c2", "conv2")):
        for lay in ("node", "edge", "out"):
            common[f"{cv}_{lay}_w1"] = p[f"{ref}_{lay}_w1"]
            common[f"{cv}_{lay}_b1"] = p[f"{ref}_{lay}_b1"].reshape(D, 1)
            common[f"{cv}_{lay}_w2"] = p[f"{ref}_{lay}_w2"]
            common[f"{cv}_{lay}_b2"] = p[f"{ref}_{lay}_b2"].reshape(1, D)
        common[f"{cv}_bn_g"] = p[f"{ref}_bn_g"].reshape(1, D)
    common["c1_norm_g"] = p["norm1_g"].reshape(1, D)
    common["c1_norm_b"] = p["norm1_b"].reshape(1, D)
    common["c2_norm_g"] = p["norm2_g"].reshape(1, D)
    common["c2_norm_b"] = p["norm2_b"].reshape(1, D)
    for pn in ("q", "k"):
        common[f"{pn}_w"] = p[f"{pn}_w"]
        common[f"{pn}_b"] = p[f"{pn}_b"].reshape(1, D)
    common["v_w"] = p["v_w"]
    common["v_b"] = p["v_b"].reshape(1, D)
    common["out_w"] = p["out_w"]
    common["out_b"] = p["out_b"].reshape(1, D)

    in_maps = []
    for c in range(NCORES):
        m = dict(common)
        m["x_ownT"] = x_ownT[c]
        m["efT"] = ef_s[c]
        m["rowT"] = rowT[c]
        m["lcolT"] = lcolT[c]
        m["invc"] = invc[c]
        in_maps.append({k: np.ascontiguousarray(v, dtype=v.dtype)
                        for k, v in m.items()})

    iters = int(os.environ.get("KERNEL_ITERS", "1"))
    results, exec_ns = _run_pjrt(nc, in_maps, iters=iters)
    LAST_EXEC_TIME_NS[0] = exec_ns
    if int(os.environ.get("KERNEL_DEBUG", "0")):
        kernel.last_debug = results
    out = np.concatenate([results[c]["out"] for c in range(NCORES)], axis=0)
    return out.reshape(16, 512, D).astype(np.float32)
